# revision 10
# baseline (speedup 1.0000x reference)
"""GPT self-attention layer (B=2, S=2048, D=1024, H=16, hd=64) on 8 TRN2 cores.

Sharding: data-parallel over batch (2) x tensor-parallel over heads (4 groups
of 4 heads). Core c handles batch b=c//4, head group g=c%4.

Per-core pipeline (bf16 attention, fp8-DoubleRow projections):
  1. QKV projections per 512-token block: fp8e4m3 x/W inputs with DoubleRow
     matmuls (4x PE rate) for blocks 1-3; block 0 takes a bf16 side path
     (xt0/wq0/wk0/wv0 inputs) because the ~3.5% fp8 x*W value error is
     amplified by the short causal softmax rows that all live there. fp8
     weights are host-prescaled by 64 (subnormal dodge) and descaled for
     free in the ACT bias op / DVE V-store. q/k/v stored bf16; V carries an
     interleaved ones column so the softmax denominator accumulates in ctx
     psum row 64 for free.
  2. Attention per head pair, scoresT [k-part, q-free], exp on ACT into
     bf16 pT (kc-pair tiles), causal diag masked by a DVE multiply, ctx
     accumulated per kc. 1/den via DVE reciprocal (row 64), broadcast
     across partitions by a matmul against a ones-at-row-64 column, DVE
     normalize -> bf16.
  3. Emission is software-pipelined three ways: scores(chunk i+1) go out
     before ctx(chunk i) so ctx never blocks the ACT exp stream; the QKV
     block loop is interleaved with pair-0 attention; and pair-1 attention
     is staggered one q-block behind pair-0 so its exp bursts fill the ACT
     gaps each QKV block creates.
  4. Four bf16 AllToAlls (one per local head, 0.5MB each) across all 8
     cores, shards duplicated across batch halves. Within each (pair, qt)
     the ctx matmuls run head-serial so the two heads' normalize+sends
     stagger, pipelining the four collectives on the CC engine (gpsimd
     holds only collectives, so the waiting triggers stall nothing).
     Receiver picks its batch half with a DVE mask-select driven by the
     bsel input (no gather).
  5. out = ctxT.T @ Wo + bo in bf16, Wo host-permuted to the per-head
     receive order: heads 0-2 partial sums plus keep-warm matmuls (HAM
     re-throttles after >3us PE idle) run under the later AllToAlls; only
     head 3's partials remain after the last one; DMA to y [512,1024] fp32.
"""

import contextlib
import ctypes
import sys
import types

sys.path.insert(0, "/opt/trn_rl_repo")

import numpy as np
import ml_dtypes

import concourse.bass as bass
import concourse.mybir as mybir
import concourse.tile as tile
from concourse import bacc
from concourse import bass_utils

P = 128
B, S, D = 2, 2048, 1024
NH_LOC = 4          # heads per core
HD = 64             # head dim
G = NH_LOC * HD     # local head dims = 256
MC = G // P         # m-chunks of local dims = 2
DC = D // P         # d-chunks = 8
TB = 512            # token block (output tokens per core, q-tile width)
NQT = S // TB       # q-tiles = 4
NTC = S // P        # token chunks = 16
NC = 8
VP = 80             # per-head v block stride (64 data + 1 ones + 15 pad)

F32 = mybir.dt.float32
F32R = mybir.dt.float32r
BF16 = mybir.dt.bfloat16
F8 = mybir.dt.float8e4
Exp = mybir.ActivationFunctionType.Exp
Ident = mybir.ActivationFunctionType.Identity
MULT = mybir.AluOpType.mult
ADD = mybir.AluOpType.add
DR = mybir.MatmulPerfMode.DoubleRow

# per-stage dtype switches (bf16 fallback for numerics bisection).
# FP8_QKV runs the q/k/v projections in fp8 DoubleRow (4x PE) for token
# blocks 1-3; block 0 always takes a bf16 side path because the fp8 x*W
# value error (~3.5%) is amplified by the short causal softmax rows that
# all live in the first block.
FP8_QKV = True
FP8_SC = True
FP8_CTX = False
WSC = 64.0 if FP8_QKV else 1.0  # host weight prescale

DX = F8 if FP8_QKV else BF16    # xt, wq/wk/wv
DQK = F8 if FP8_SC else BF16    # stored q / k
DP = F8 if FP8_CTX else BF16    # pT probs and v

NP_BF16 = np.dtype(ml_dtypes.bfloat16)
NP_F8 = np.dtype(ml_dtypes.float8_e4m3)


def _install_ntff_hook():
    """Make trace=True work under axon: inject antenv.axon_hooks backed by
    ctypes calls into libaxon_pjrt.so (mirrors trn_agent_boot logic)."""
    if "antenv.axon_hooks" in sys.modules:
        return
    holder = {}
    mod = types.ModuleType("antenv.axon_hooks")
    mod.set_axon_ntff_profile_hook = lambda h: holder.update(h=h)
    mod.get_axon_ntff_profile_hook = lambda: holder.get("h")
    sys.modules["antenv.axon_hooks"] = mod
    try:
        lib = ctypes.CDLL("/opt/axon/libaxon_pjrt.so")
        if not hasattr(lib, "axon_start_nrt_profile"):
            return
    except OSError:
        return
    lib.axon_start_nrt_profile.argtypes = [
        ctypes.POINTER(ctypes.c_int64),
        ctypes.c_size_t,
    ]
    lib.axon_start_nrt_profile.restype = ctypes.c_int64
    lib.axon_stop_nrt_profile.argtypes = [ctypes.c_char_p]
    lib.axon_stop_nrt_profile.restype = ctypes.c_int64

    @contextlib.contextmanager
    def _hook(output_dir, device_ids):
        import jax

        jax.devices()
        if device_ids:
            ids = (ctypes.c_int64 * len(device_ids))(*device_ids)
            rc = lib.axon_start_nrt_profile(ids, len(device_ids))
        else:
            rc = lib.axon_start_nrt_profile(None, 0)
        if rc != 0:
            raise RuntimeError(f"axon_start_nrt_profile rc={rc}")
        try:
            yield
        finally:
            n = lib.axon_stop_nrt_profile(str(output_dir).encode())
            print(f"profile: {n} ntff file(s) written to {output_dir}")

    holder["h"] = _hook


def build(coll=True):
    nc = bacc.Bacc("TRN2", target_bir_lowering=False, debug=False, num_devices=NC)

    xt_d = nc.dram_tensor("xt", [D, S], DX, kind="ExternalInput").ap()
    wq_d = nc.dram_tensor("wq", [D, G], DX, kind="ExternalInput").ap()
    wk_d = nc.dram_tensor("wk", [D, G], DX, kind="ExternalInput").ap()
    wv_d = nc.dram_tensor("wv", [D, G], DX, kind="ExternalInput").ap()
    xt0_d = nc.dram_tensor("xt0", [D, TB], BF16, kind="ExternalInput").ap()
    wq0_d = nc.dram_tensor("wq0", [D, G], BF16, kind="ExternalInput").ap()
    wk0_d = nc.dram_tensor("wk0", [D, G], BF16, kind="ExternalInput").ap()
    wv0_d = nc.dram_tensor("wv0", [D, G], BF16, kind="ExternalInput").ap()
    bq_d = nc.dram_tensor("bq", [P, MC], F32, kind="ExternalInput").ap()
    bk_d = nc.dram_tensor("bk", [P, MC], F32, kind="ExternalInput").ap()
    bv_d = nc.dram_tensor("bv", [1, G], F32, kind="ExternalInput").ap()
    wo_d = nc.dram_tensor("wo", [D, D], BF16, kind="ExternalInput").ap()
    bo_d = nc.dram_tensor("bo", [1, D], F32, kind="ExternalInput").ap()
    bsel_d = nc.dram_tensor("bsel", [P, 2], F32, kind="ExternalInput").ap()
    y_d = nc.dram_tensor("y", [TB, D], F32, kind="ExternalOutput").ap()

    with tile.TileContext(nc) as tc:
        with (
            tc.tile_pool(name="const", bufs=1) as const,
            tc.tile_pool(name="dram", bufs=1, space="DRAM") as dram,
            tc.tile_pool(name="ps_mm", bufs=2, space="PSUM") as ps_mm,
            tc.tile_pool(name="ps_ctx", bufs=4, space="PSUM") as ps_ctx,
            tc.tile_pool(name="persist", bufs=1) as persist,
        ):
            # ---------------- constants ----------------
            ones_f = const.tile([P, 1], F32, tag="ones_f")
            nc.vector.memset(ones_f[:], 1.0)
            # trimask[k, u] = 1 if k <= u else 0 (keep where u - k >= 0)
            tri_f = const.tile([P, P], F32, tag="tri_f")
            nc.gpsimd.memset(tri_f[:], 1.0)
            nc.gpsimd.affine_select(
                out=tri_f[:],
                in_=tri_f[:],
                compare_op=mybir.AluOpType.is_ge,
                fill=0.0,
                base=0,
                pattern=[[1, P]],
                channel_multiplier=-1,
            )
            # materialized for both heads so the gpsimd mask multiply uses a
            # plain strided AP (no broadcast)
            tri_p = const.tile([P, 2, P], DP, tag="tri_p")
            nc.vector.tensor_copy(
                tri_p[:], tri_f[:, None, :].to_broadcast((P, 2, P))
            )
            # ones at row 64 only: broadcast-den matmul weights
            zrow_f = const.tile([P, HD], F32, tag="zrow_f")
            nc.vector.memset(zrow_f[:], 0.0)
            nc.vector.memset(zrow_f[64:65, :], 1.0)
            onescol_r = const.tile([P, HD], F32R, tag="onescol_r")
            nc.vector.tensor_copy(onescol_r[:], zrow_f[:])
            zeros_f = const.tile([P, 512], F32, tag="zeros_f")
            nc.vector.memset(zeros_f[:], 0.0)

            bq_sb = const.tile([P, MC], F32, tag="bq")
            bk_sb = const.tile([P, MC], F32, tag="bk")
            nc.sync.dma_start(bq_sb[:], bq_d)
            nc.sync.dma_start(bk_sb[:], bk_d)
            bv_row = const.tile([1, G], F32, tag="bv_row")
            nc.sync.dma_start(bv_row[:], bv_d)
            bv_bc = const.tile([P, G], F32, tag="bv_bc")
            nc.gpsimd.partition_broadcast(bv_bc[:], bv_row[:])
            bo_row = const.tile([1, D], F32, tag="bo_row")
            bo_bc = const.tile([P, D], F32, tag="bo_bc")
            bsel_sb = const.tile([P, 2], F32, tag="bsel")
            nc.sync.dma_start(bsel_sb[:], bsel_d)

            # persistent activations: q/k [p, pair, kslab, tok] with kslab 1
            # zeroed (DoubleRow zero-pad), v [p, tc, head, 80] fp8
            KSL = 2 if FP8_SC else 1
            qT = persist.tile([P, MC, KSL, S], DQK, tag="qT")
            kT = persist.tile([P, MC, KSL, S], DQK, tag="kT")
            v_sb = persist.tile([P, NTC, NH_LOC, VP], DP, tag="v")
            wo_sb = persist.tile([P, DC, D], BF16, tag="wo")

            if FP8_SC:
                nc.gpsimd.memset(qT[:, :, 1, :], 0.0)
                nc.gpsimd.memset(kT[:, :, 1, :], 0.0)
            # ones column of v (denominator trick): col 64 of each head block
            nc.gpsimd.memset(v_sb[:, :, :, HD : HD + 1], 1.0)

            a2a_in = [
                dram.tile([NC * HD, TB], BF16, name=f"a2ain{h}", tag=f"a2ain{h}")
                for h in range(NH_LOC)
            ]
            a2a_out = [
                dram.tile([NC * HD, TB], BF16, name=f"a2aout{h}", tag=f"a2aout{h}")
                for h in range(NH_LOC)
            ]

            with (
                tc.tile_pool(name="xw", bufs=1) as xw,
            ):
                wq_sb = xw.tile([P, DC, G], DX, tag="wq")
                wk_sb = xw.tile([P, DC, G], DX, tag="wk")
                wv_sb = xw.tile([P, DC, G], DX, tag="wv")
                # bf16 block-0 path: loaded first since QKV(0) starts the
                # whole pipeline
                wq0_sb = xw.tile([P, DC, G], BF16, tag="wq0")
                wk0_sb = xw.tile([P, DC, G], BF16, tag="wk0")
                wv0_sb = xw.tile([P, DC, G], BF16, tag="wv0")
                x0_sb = xw.tile([P, DC, TB], BF16, tag="x0")
                # per-dc-chunk DMAs: the first q matmul chain only waits for
                # the dc slices it has consumed so far, not the whole tile
                wq0_r = wq0_d.rearrange("(dc p) m -> p dc m", p=P)
                x0_r = xt0_d.rearrange("(dc p) t -> p dc t", p=P)
                for dc in range(DC):
                    nc.sync.dma_start(wq0_sb[:, dc, :], wq0_r[:, dc, :])
                    nc.sync.dma_start(x0_sb[:, dc, :], x0_r[:, dc, :])
                nc.sync.dma_start(
                    wk0_sb[:], wk0_d.rearrange("(dc p) m -> p dc m", p=P)
                )
                nc.sync.dma_start(
                    wv0_sb[:], wv0_d.rearrange("(dc p) m -> p dc m", p=P)
                )
                nc.sync.dma_start(wq_sb[:], wq_d.rearrange("(dc p) m -> p dc m", p=P))

                xTt = [
                    xw.tile([P, DC, TB], DX, tag=f"xT{g}", name=f"xT{g}")
                    for g in range(1, NQT)
                ]
                xTg = [None] + [t[:] for t in xTt]
                xt_r = xt_d.rearrange("(dc p) t -> p dc t", p=P)

                for g in range(1, NQT):
                    nc.sync.dma_start(
                        xTg[g], xt_r[:, :, g * TB : (g + 1) * TB]
                    )
                    if g == 1:
                        nc.sync.dma_start(
                            wk_sb[:], wk_d.rearrange("(dc p) m -> p dc m", p=P)
                        )
                        nc.sync.dma_start(
                            wv_sb[:], wv_d.rearrange("(dc p) m -> p dc m", p=P)
                        )

                def emit_qkv(g):
                    if FP8_QKV and g > 0:
                        wq_g, wk_g, wv_g, xg = wq_sb, wk_sb, wv_sb, xTg[g]
                        sc, use_dr = 1.0 / WSC, True
                    else:
                        wq_g, wk_g, wv_g, xg = wq0_sb, wk0_sb, wv0_sb, x0_sb[:]
                        sc, use_dr = 1.0, False
                    for w_sb, b_sb, out_t in ((wq_g, bq_sb, qT), (wk_g, bk_sb, kT)):
                        for mc_i in range(MC):
                            pj = ps_mm.tile(
                                [P, 2, 512], F32, tag="mm", name="pj"
                            )[:, 0, :]
                            if use_dr:
                                for i, dc in enumerate(range(0, DC, 2)):
                                    nc.tensor.matmul(
                                        pj[:],
                                        w_sb[:, dc : dc + 2, mc_i * P : (mc_i + 1) * P],
                                        xg[:, dc : dc + 2, :],
                                        start=(i == 0),
                                        stop=(dc == DC - 2),
                                        perf_mode=DR,
                                    )
                            else:
                                for dc in range(DC):
                                    nc.tensor.matmul(
                                        pj[:],
                                        w_sb[:, dc, mc_i * P : (mc_i + 1) * P],
                                        xg[:, dc, :],
                                        start=(dc == 0),
                                        stop=(dc == DC - 1),
                                    )
                            nc.scalar.activation(
                                out_t[:, mc_i, 0, g * TB : (g + 1) * TB],
                                pj[:],
                                Ident,
                                bias=b_sb[:, mc_i : mc_i + 1],
                                scale=sc,
                            )
                    for ti in range(4):
                        tc_i = 4 * g + ti
                        pv = ps_mm.tile(
                            [P, 2, 512], F32, tag="mm", name="pv"
                        )[:, 0, :]
                        if use_dr:
                            for i, dc in enumerate(range(0, DC, 2)):
                                nc.tensor.matmul(
                                    pv[:, 0:G],
                                    xg[:, dc : dc + 2, ti * P : (ti + 1) * P],
                                    wv_g[:, dc : dc + 2, :],
                                    start=(i == 0),
                                    stop=(dc == DC - 2),
                                    perf_mode=DR,
                                )
                        else:
                            for dc in range(DC):
                                nc.tensor.matmul(
                                    pv[:, 0:G],
                                    xg[:, dc, ti * P : (ti + 1) * P],
                                    wv_g[:, dc, :],
                                    start=(dc == 0),
                                    stop=(dc == DC - 1),
                                )
                        # v = pv*sc + bv into padded head blocks
                        nc.vector.scalar_tensor_tensor(
                            v_sb[:, tc_i, :, 0:HD],
                            pv[:, 0:G].rearrange("p (h c) -> p h c", c=HD),
                            sc,
                            bv_bc[:].rearrange("p (h c) -> p h c", c=HD),
                            MULT,
                            ADD,
                        )

                # ------ attention, emission-interleaved with QKV blocks ------
                nc.sync.dma_start(
                    wo_sb[:], wo_d.rearrange("(dc p) n -> p dc n", p=P)
                )
                nc.sync.dma_start(bo_row[:], bo_d)
                nc.gpsimd.partition_broadcast(bo_bc[:], bo_row[:])
                work = tc.alloc_tile_pool(name="att", bufs=1)
                pTp = tc.alloc_tile_pool(name="pTp", bufs=8)
                smallp = tc.alloc_tile_pool(name="smallp", bufs=3)
                ctxn = [
                    [
                        work.tile(
                            [HD, TB], BF16, tag=f"ctxn{h}_{q}", name=f"ctxn{h}_{q}"
                        )
                        for q in range(NQT)
                    ]
                    for h in range(NH_LOC)
                ]
                # reciprocal of denominator lives at row 64; other rows stay 0
                # so the broadcast matmul (ones at row 64) reads no garbage
                rdenX = [
                    work.tile([P, 512], F32, tag=f"rdenX{i}", name=f"rdenX{i}")
                    for i in range(2)
                ]
                for i in range(2):
                    nc.vector.tensor_copy(rdenX[i][:], zeros_f[:])
                c_ps_of = {}

                def emit_scores(pair, ch):
                    qt, k0, k1 = ch
                    p_tiles = {}
                    for kc in range(k0, k1):
                        j = kc - 4 * qt
                        coff = max(0, j) * P
                        if kc % 2 == 0:
                            pT = pTp.tile([P, 2, 2, TB], DP, tag="pT")
                            p_tiles[kc] = pT
                        else:
                            pT = p_tiles[kc - 1]
                        s_ps = ps_mm.tile([P, 2, 512], F32, tag="mm")
                        for h01 in range(2):
                            pb = h01 * HD
                            if FP8_SC:
                                nc.tensor.matmul(
                                    s_ps[:, h01, coff:512],
                                    kT[pb : pb + HD, pair, :, kc * P : (kc + 1) * P],
                                    qT[
                                        pb : pb + HD,
                                        pair,
                                        :,
                                        qt * TB + coff : (qt + 1) * TB,
                                    ],
                                    start=True,
                                    stop=True,
                                    perf_mode=DR,
                                )
                            else:
                                nc.tensor.matmul(
                                    s_ps[:, h01, coff:512],
                                    kT[pb : pb + HD, pair, 0, kc * P : (kc + 1) * P],
                                    qT[
                                        pb : pb + HD,
                                        pair,
                                        0,
                                        qt * TB + coff : (qt + 1) * TB,
                                    ],
                                    start=True,
                                    stop=True,
                                )
                        nc.scalar.activation(
                            pT[:, kc % 2, :, coff:512],
                            s_ps[:, :, coff:512],
                            Exp,
                            scale=0.125,
                        )
                        if j >= 0:
                            nc.vector.tensor_tensor(
                                pT[:, kc % 2, :, coff : coff + P],
                                pT[:, kc % 2, :, coff : coff + P],
                                tri_p[:],
                                MULT,
                            )
                    return p_tiles

                def emit_ctx(pair, ch, p_tiles, tail=False):
                    qt, k0, k1 = ch
                    nkc = 4 * qt + 4
                    if k0 == 0:
                        c_ps_of[pair, qt] = [
                            ps_ctx.tile([P, 512], F32, tag="ctx", name=f"cps{h01}")
                            for h01 in range(2)
                        ]
                    c_ps = c_ps_of[pair, qt]
                    final = k1 == nkc
                    # head-serial: head h01's ctx, normalize, sends (and on
                    # qt3 its collective) all go out before head h01+1's ctx,
                    # staggering the per-head AllToAlls on the CC engine
                    for h01 in range(2):
                        h = 2 * pair + h01
                        for kc in range(k0, k1):
                            j = kc - 4 * qt
                            pT = p_tiles[kc - kc % 2]
                            coff = max(0, j) * P
                            nc.tensor.matmul(
                                c_ps[h01][0 : HD + 1, coff:512],
                                v_sb[:, kc, h, 0 : HD + 1],
                                pT[:, kc % 2, h01, coff:512],
                                start=(kc == 0),
                                stop=(kc == nkc - 1),
                            )
                        if not final:
                            continue
                        # qt complete: normalize (divide by the ones-row
                        # sums, broadcast across partitions via a row-64
                        # matmul)
                        rX = rdenX[h01]
                        if tail and h01 == 1:
                            # final flush gates the last collective trigger
                            # and ACT is idle: run the second head's
                            # reciprocal there so both proceed in parallel
                            lnd = smallp.tile([P, 512], F32, tag="bb")
                            nc.scalar.activation(
                                lnd[64:65, :],
                                c_ps[h01][64:65, :],
                                mybir.ActivationFunctionType.Ln,
                            )
                            nc.scalar.activation(
                                rX[64:65, :], lnd[64:65, :], Exp, scale=-1.0
                            )
                        else:
                            with nc.allow_low_precision(reason="den recip"):
                                nc.vector.reciprocal(
                                    rX[64:65, :], c_ps[h01][64:65, :]
                                )
                        b_ps = ps_mm.tile([P, 2, 512], F32, tag="mm", name="bps")[
                            :, 0, :
                        ]
                        nc.tensor.matmul(
                            b_ps[0:HD, :],
                            zrow_f[:, 0:HD],
                            rX[:],
                            start=True,
                            stop=True,
                        )
                        bb = smallp.tile([HD, 512], F32, tag="bb")
                        nc.scalar.copy(bb[:], b_ps[0:HD, :])
                        nc.vector.tensor_tensor(
                            ctxn[h][qt][:, :],
                            c_ps[h01][0:HD, :],
                            bb[:],
                            MULT,
                        )
                        # A2A sends: destination block qt, duplicated
                        # across batch halves
                        for sh in (qt, qt + 4):
                            nc.sync.dma_start(
                                a2a_in[h][sh * HD : (sh + 1) * HD, :],
                                ctxn[h][qt][:, :],
                            )
                        if qt == NQT - 1 and coll:
                            nc.gpsimd.collective_compute(
                                "AllToAll",
                                mybir.AluOpType.bypass,
                                ins=[a2a_in[h].opt()],
                                outs=[a2a_out[h].opt()],
                                replica_groups=[list(range(NC))],
                            )

                def qt_chunks(qt):
                    return [
                        (qt, kcb, min(kcb + 8, 4 * qt + 4))
                        for kcb in range(0, 4 * qt + 4, 8)
                    ]

                # software pipeline: emit scores(i+1) before ctx(i) so the PE
                # queue never blocks the ACT exp stream behind ctx matmuls;
                # pair-0 attention interleaves with the QKV q-block loop
                pend = []

                def push(pair, ch):
                    tiles = emit_scores(pair, ch)
                    if pend:
                        emit_ctx(*pend.pop())
                    pend.append((pair, ch, tiles))

                # pair-1 staggered one q-block behind pair-0: its exp
                # bursts fill the ACT gaps left by each QKV block
                for g in range(NQT):
                    emit_qkv(g)
                    for ch in qt_chunks(g):
                        push(0, ch)
                    if g == NQT - 1:
                        # force-flush pair-0 qt3's ctx+norm before pair-1
                        # qt2 scores so A2A(h0)/A2A(h1) fire ~10us earlier
                        emit_ctx(*pend.pop())
                    if g >= 1:
                        for ch in qt_chunks(g - 1):
                            push(1, ch)
                emit_ctx(*pend.pop())
                for ch in qt_chunks(NQT - 1):
                    push(1, ch)
                _fp, _fc, _ft = pend.pop()
                emit_ctx(_fp, _fc, _ft, tail=True)

                smallp.release()
                pTp.release()
                work.release()

            # ---------- receive + output projection ----------
            # per-head receive: a2a_out[h] rows = 8 senders x 64 dims of
            # local head h for my token block; jj blocks of 128 rows pair
            # senders (2jj, 2jj+1), jj 0-1 batch 0, jj 2-3 batch 1
            outp = tc.alloc_tile_pool(name="outp", bufs=1)
            gsrc = a2a_out if coll else a2a_in
            cx = outp.tile([P, NH_LOC, 4, TB], BF16, tag="cx")
            tmp = outp.tile([P, 2, TB], BF16, tag="seltmp")
            # ctxf[p, h, jj, t]: wo_sb chunk h*2+jj (Wo rows host-permuted)
            ctxf = outp.tile([P, NH_LOC, 2, TB], BF16, tag="ctxf")

            def recv_head(h):
                # readback + batch select (ctxf = lo*bsel[0] + hi*bsel[1])
                # per column half: the first po chains start on half the data
                for hh in range(2):
                    hc = slice(hh * 256, (hh + 1) * 256)
                    nc.sync.dma_start(
                        cx[:, h, :, hc],
                        gsrc[h][:, hc].rearrange("(j q) t -> q j t", q=P),
                    )
                    nc.vector.tensor_scalar(
                        tmp[:, :, hc], cx[:, h, 0:2, hc], bsel_sb[:, 0:1],
                        None, MULT,
                    )
                    nc.vector.scalar_tensor_tensor(
                        ctxf[:, h, :, hc], cx[:, h, 2:4, hc],
                        bsel_sb[:, 1:2], tmp[:, :, hc], MULT, ADD,
                    )

            with tc.tile_pool(name="out_pool", bufs=3) as out_pool:
                o_parts = [
                    outp.tile([P, 512], F32, tag=f"opart{u}", name=f"opart{u}")
                    for u in range(8)
                ]
                # heads 0-1 partials: overlap with A2As of heads 2-3
                recv_head(0)
                recv_head(1)
                for u in range(8):
                    tc_i, nt = u // 2, u % 2
                    po = ps_mm.tile([P, 2, 512], F32, tag="mm")
                    for i, (h, jj) in enumerate(
                        [(0, 0), (0, 1), (1, 0), (1, 1)]
                    ):
                        nc.tensor.matmul(
                            po[:, 0, :],
                            ctxf[:, h, jj, tc_i * P : (tc_i + 1) * P],
                            wo_sb[:, h * 2 + jj, nt * 512 : (nt + 1) * 512],
                            start=(i == 0),
                            stop=(i == 3),
                        )
                    nc.vector.tensor_tensor(
                        o_parts[u][:],
                        po[:, 0, :],
                        bo_bc[:, nt * 512 : (nt + 1) * 512],
                        ADD,
                    )
                # head-2 partials accumulate into o_parts under A2A of head 3
                recv_head(2)
                for u in range(8):
                    tc_i, nt = u // 2, u % 2
                    po = ps_mm.tile([P, 2, 512], F32, tag="mm")
                    for jj in range(2):
                        nc.tensor.matmul(
                            po[:, 0, :],
                            ctxf[:, 2, jj, tc_i * P : (tc_i + 1) * P],
                            wo_sb[:, 4 + jj, nt * 512 : (nt + 1) * 512],
                            start=(jj == 0),
                            stop=(jj == 1),
                        )
                    nc.vector.tensor_tensor(
                        o_parts[u][:], po[:, 0, :], o_parts[u][:], ADD
                    )
                # keep-warm: a few dummy matmuls bridging to the last A2A so
                # the PE doesn't re-throttle (>3us idle drops HAM to half
                # clock) before the head-3 output projection
                warm = ps_mm.tile([P, 2, 512], F32, tag="mm", name="warm")
                for wj in range(12):
                    nc.tensor.matmul(
                        warm[:, 0, :],
                        ctxf[:, 0, wj % 2, 0:P],
                        wo_sb[:, 0, 0:512],
                        start=True,
                        stop=True,
                    )
                recv_head(3)
                for u in range(8):
                    tc_i, nt = u // 2, u % 2
                    po = ps_mm.tile([P, 2, 512], F32, tag="mm")
                    for jj in range(2):
                        nc.tensor.matmul(
                            po[:, 0, :],
                            ctxf[:, 3, jj, tc_i * P : (tc_i + 1) * P],
                            wo_sb[:, 6 + jj, nt * 512 : (nt + 1) * 512],
                            start=(jj == 0),
                            stop=(jj == 1),
                        )
                    o_sb = out_pool.tile([P, 512], F32, tag="osb")
                    nc.vector.tensor_tensor(
                        o_sb[:], po[:, 0, :], o_parts[u][:], ADD
                    )
                    nc.sync.dma_start(
                        y_d[
                            tc_i * P : (tc_i + 1) * P,
                            nt * 512 : (nt + 1) * 512,
                        ],
                        o_sb[:],
                    )

            outp.release()

    nc.compile()
    return nc


_NC_CACHE = {}


def _get_nc():
    if "nc" not in _NC_CACHE:
        _NC_CACHE["nc"] = build()
    return _NC_CACHE["nc"]


def _make_in_maps(x, Wq, bq, Wk, bk, Wv, bv, Wo, bo):
    x = np.asarray(x, np.float32)
    Wq, Wk, Wv, Wo = (np.asarray(a, np.float32) for a in (Wq, Wk, Wv, Wo))
    bq, bk, bv, bo = (np.asarray(a, np.float32) for a in (bq, bk, bv, bo))
    np_dx = NP_F8 if FP8_QKV else NP_BF16
    # permute Wo rows to the per-head receive order: sbuf chunk c = h*2+jj,
    # partition p holds sender group g' = 2*jj + p//64's head h, dim p%64
    # (global head g'*4 + h)
    perm = np.empty(D, np.int64)
    for ci in range(DC):
        h, jj = ci // 2, ci % 2
        for par in range(2):
            H = (2 * jj + par) * 4 + h
            base = ci * P + par * HD
            perm[base : base + HD] = np.arange(H * HD, (H + 1) * HD)
    wo_b = np.ascontiguousarray(Wo[perm]).astype(NP_BF16)
    in_maps = []
    for c in range(NC):
        b, g = c // 4, c % 4
        sl = slice(g * G, (g + 1) * G)
        bsel = np.tile(
            np.array([1.0 - b, float(b)], np.float32).reshape(1, 2), (P, 1)
        )
        in_maps.append(
            {
                "xt": np.ascontiguousarray(x[b].T).astype(np_dx),
                "wq": np.ascontiguousarray(Wq[:, sl] * WSC).astype(np_dx),
                "wk": np.ascontiguousarray(Wk[:, sl] * WSC).astype(np_dx),
                "wv": np.ascontiguousarray(Wv[:, sl] * WSC).astype(np_dx),
                "xt0": np.ascontiguousarray(x[b].T[:, :TB]).astype(NP_BF16),
                "wq0": np.ascontiguousarray(Wq[:, sl]).astype(NP_BF16),
                "wk0": np.ascontiguousarray(Wk[:, sl]).astype(NP_BF16),
                "wv0": np.ascontiguousarray(Wv[:, sl]).astype(NP_BF16),
                "bq": np.ascontiguousarray(bq[sl].reshape(MC, P).T),
                "bk": np.ascontiguousarray(bk[sl].reshape(MC, P).T),
                "bv": np.ascontiguousarray(bv[sl].reshape(1, G)),
                "wo": wo_b,
                "bo": np.ascontiguousarray(bo.reshape(1, D)),
                "bsel": np.ascontiguousarray(bsel),
            }
        )
    return in_maps


def run(inputs, trace=False, tmpdir=None):
    """Run on 8 cores; returns (output [2,2048,1024], BassKernelResults)."""
    if trace:
        _install_ntff_hook()
    nc = _get_nc()
    in_maps = _make_in_maps(**inputs)
    res = bass_utils.run_bass_kernel_spmd(
        nc, in_maps, core_ids=list(range(NC)), trace=trace, tmpdir=tmpdir
    )
    out = np.empty((B, S, D), np.float32)
    for c in range(NC):
        b, g = c // 4, c % 4
        out[b, g * TB : (g + 1) * TB, :] = res.results[c]["y"]
    return out, res


def kernel(**inputs) -> np.ndarray:
    out, _ = run(inputs, trace=False)
    return out



# revision 15
# speedup vs baseline: 1.1173x; 1.1173x over previous
"""GPT self-attention layer (B=2, S=2048, D=1024, H=16, hd=64) on 8 TRN2 cores.

Sharding: data-parallel over batch (2) x tensor-parallel over heads (4 groups
of 4 heads). Core c handles batch b=c//4, head group g=c%4.

Per-core pipeline (bf16 attention, fp8-DoubleRow projections):
  1. QKV projections per 512-token block: fp8e4m3 x/W inputs with DoubleRow
     matmuls (4x PE rate) for blocks 1-3; block 0 takes a bf16 side path
     (xt0/wq0/wk0/wv0 inputs) because the ~3.5% fp8 x*W value error is
     amplified by the short causal softmax rows that all live there. fp8
     weights are host-prescaled by 64 (subnormal dodge) and descaled for
     free in the ACT bias op / DVE V-store. q/k/v stored bf16; V carries an
     interleaved ones column so the softmax denominator accumulates in ctx
     psum row 64 for free.
  2. Attention per head pair, scoresT [k-part, q-free], exp on ACT into
     bf16 pT (kc-pair tiles), causal diag masked by a DVE multiply, ctx
     accumulated per kc. 1/den via DVE reciprocal (row 64), broadcast
     across partitions by a matmul against a ones-at-row-64 column, DVE
     normalize -> bf16.
  3. Emission is software-pipelined three ways: scores(chunk i+1) go out
     before ctx(chunk i) so ctx never blocks the ACT exp stream; the QKV
     block loop is interleaved with pair-0 attention; and pair-1 attention
     is staggered one q-block behind pair-0 so its exp bursts fill the ACT
     gaps each QKV block creates.
  4. Four bf16 AllToAlls (one per local head, 0.5MB each) across all 8
     cores, shards duplicated across batch halves. Within each (pair, qt)
     the ctx matmuls run head-serial so the two heads' normalize+sends
     stagger, pipelining the four collectives on the CC engine (gpsimd
     holds only collectives, so the waiting triggers stall nothing).
     Receiver picks its batch half with a DVE mask-select driven by the
     bsel input (no gather).
  5. out = ctxT.T @ Wo + bo in bf16, Wo host-permuted to the per-head
     receive order: heads 0-2 partial sums plus keep-warm matmuls (HAM
     re-throttles after >3us PE idle) run under the later AllToAlls; only
     head 3's partials remain after the last one; DMA to y [512,1024] fp32.
"""

import contextlib
import ctypes
import sys
import types

sys.path.insert(0, "/opt/trn_rl_repo")

import numpy as np
import ml_dtypes

import concourse.bass as bass
import concourse.mybir as mybir
import concourse.tile as tile
from concourse import bacc
from concourse import bass_utils

P = 128
B, S, D = 2, 2048, 1024
NH_LOC = 4          # heads per core
HD = 64             # head dim
G = NH_LOC * HD     # local head dims = 256
MC = G // P         # m-chunks of local dims = 2
DC = D // P         # d-chunks = 8
TB = 512            # token block (output tokens per core, q-tile width)
NQT = S // TB       # q-tiles = 4
NTC = S // P        # token chunks = 16
NC = 8
VP = 80             # per-head v block stride (64 data + 1 ones + 15 pad)

F32 = mybir.dt.float32
F32R = mybir.dt.float32r
BF16 = mybir.dt.bfloat16
F8 = mybir.dt.float8e4
Exp = mybir.ActivationFunctionType.Exp
Ident = mybir.ActivationFunctionType.Identity
MULT = mybir.AluOpType.mult
ADD = mybir.AluOpType.add
DR = mybir.MatmulPerfMode.DoubleRow

# per-stage dtype switches (bf16 fallback for numerics bisection).
# FP8_QKV runs the q/k/v projections in fp8 DoubleRow (4x PE) for token
# blocks 1-3; block 0 always takes a bf16 side path because the fp8 x*W
# value error (~3.5%) is amplified by the short causal softmax rows that
# all live in the first block.
FP8_QKV = True
FP8_SC = True
FP8_CTX = False
WSC = 64.0 if FP8_QKV else 1.0  # host weight prescale

DX = F8 if FP8_QKV else BF16    # xt, wq/wk/wv
DQK = F8 if FP8_SC else BF16    # stored q / k
DP = F8 if FP8_CTX else BF16    # pT probs and v

NP_BF16 = np.dtype(ml_dtypes.bfloat16)
NP_F8 = np.dtype(ml_dtypes.float8_e4m3)


def _install_ntff_hook():
    """Make trace=True work under axon: inject antenv.axon_hooks backed by
    ctypes calls into libaxon_pjrt.so (mirrors trn_agent_boot logic)."""
    if "antenv.axon_hooks" in sys.modules:
        return
    holder = {}
    mod = types.ModuleType("antenv.axon_hooks")
    mod.set_axon_ntff_profile_hook = lambda h: holder.update(h=h)
    mod.get_axon_ntff_profile_hook = lambda: holder.get("h")
    sys.modules["antenv.axon_hooks"] = mod
    try:
        lib = ctypes.CDLL("/opt/axon/libaxon_pjrt.so")
        if not hasattr(lib, "axon_start_nrt_profile"):
            return
    except OSError:
        return
    lib.axon_start_nrt_profile.argtypes = [
        ctypes.POINTER(ctypes.c_int64),
        ctypes.c_size_t,
    ]
    lib.axon_start_nrt_profile.restype = ctypes.c_int64
    lib.axon_stop_nrt_profile.argtypes = [ctypes.c_char_p]
    lib.axon_stop_nrt_profile.restype = ctypes.c_int64

    @contextlib.contextmanager
    def _hook(output_dir, device_ids):
        import jax

        jax.devices()
        if device_ids:
            ids = (ctypes.c_int64 * len(device_ids))(*device_ids)
            rc = lib.axon_start_nrt_profile(ids, len(device_ids))
        else:
            rc = lib.axon_start_nrt_profile(None, 0)
        if rc != 0:
            raise RuntimeError(f"axon_start_nrt_profile rc={rc}")
        try:
            yield
        finally:
            n = lib.axon_stop_nrt_profile(str(output_dir).encode())
            print(f"profile: {n} ntff file(s) written to {output_dir}")

    holder["h"] = _hook


def build(coll=True):
    nc = bacc.Bacc("TRN2", target_bir_lowering=False, debug=False, num_devices=NC)

    xt_d = nc.dram_tensor("xt", [D, S], DX, kind="ExternalInput").ap()
    wq_d = nc.dram_tensor("wq", [D, G], DX, kind="ExternalInput").ap()
    wk_d = nc.dram_tensor("wk", [D, G], DX, kind="ExternalInput").ap()
    wv_d = nc.dram_tensor("wv", [D, G], DX, kind="ExternalInput").ap()
    xt0_d = nc.dram_tensor("xt0", [D, TB], BF16, kind="ExternalInput").ap()
    wq0_d = nc.dram_tensor("wq0", [D, G], BF16, kind="ExternalInput").ap()
    wk0_d = nc.dram_tensor("wk0", [D, G], BF16, kind="ExternalInput").ap()
    wv0_d = nc.dram_tensor("wv0", [D, G], BF16, kind="ExternalInput").ap()
    bq_d = nc.dram_tensor("bq", [P, MC], F32, kind="ExternalInput").ap()
    bk_d = nc.dram_tensor("bk", [P, MC], F32, kind="ExternalInput").ap()
    bv_d = nc.dram_tensor("bv", [1, G], F32, kind="ExternalInput").ap()
    wo_d = nc.dram_tensor("wo", [D, D], BF16, kind="ExternalInput").ap()
    bo_d = nc.dram_tensor("bo", [1, D], F32, kind="ExternalInput").ap()
    bsel_d = nc.dram_tensor("bsel", [P, 2], F32, kind="ExternalInput").ap()
    y_d = nc.dram_tensor("y", [TB, D], F32, kind="ExternalOutput").ap()

    with tile.TileContext(nc) as tc:
        with (
            tc.tile_pool(name="const", bufs=1) as const,
            tc.tile_pool(name="dram", bufs=1, space="DRAM") as dram,
            tc.tile_pool(name="ps_mm", bufs=2, space="PSUM") as ps_mm,
            tc.tile_pool(name="ps_ctx", bufs=4, space="PSUM") as ps_ctx,
            tc.tile_pool(name="persist", bufs=1) as persist,
        ):
            # ---------------- constants ----------------
            ones_f = const.tile([P, 1], F32, tag="ones_f")
            nc.vector.memset(ones_f[:], 1.0)
            # trimask[k, u] = 1 if k <= u else 0 (keep where u - k >= 0)
            tri_f = const.tile([P, P], F32, tag="tri_f")
            nc.gpsimd.memset(tri_f[:], 1.0)
            nc.gpsimd.affine_select(
                out=tri_f[:],
                in_=tri_f[:],
                compare_op=mybir.AluOpType.is_ge,
                fill=0.0,
                base=0,
                pattern=[[1, P]],
                channel_multiplier=-1,
            )
            # materialized for both heads so the gpsimd mask multiply uses a
            # plain strided AP (no broadcast)
            tri_p = const.tile([P, 2, P], DP, tag="tri_p")
            nc.vector.tensor_copy(
                tri_p[:], tri_f[:, None, :].to_broadcast((P, 2, P))
            )
            # ones at row 64 only: broadcast-den matmul weights
            zrow_f = const.tile([P, HD], F32, tag="zrow_f")
            nc.vector.memset(zrow_f[:], 0.0)
            nc.vector.memset(zrow_f[64:65, :], 1.0)
            onescol_r = const.tile([P, HD], F32R, tag="onescol_r")
            nc.vector.tensor_copy(onescol_r[:], zrow_f[:])
            zeros_f = const.tile([P, 512], F32, tag="zeros_f")
            nc.vector.memset(zeros_f[:], 0.0)

            bq_sb = const.tile([P, MC], F32, tag="bq")
            bk_sb = const.tile([P, MC], F32, tag="bk")
            nc.sync.dma_start(bq_sb[:], bq_d)
            nc.sync.dma_start(bk_sb[:], bk_d)
            bv_row = const.tile([1, G], F32, tag="bv_row")
            nc.sync.dma_start(bv_row[:], bv_d)
            bv_bc = const.tile([P, G], F32, tag="bv_bc")
            nc.gpsimd.partition_broadcast(bv_bc[:], bv_row[:])
            bo_row = const.tile([1, D], F32, tag="bo_row")
            bo_bc = const.tile([P, D], F32, tag="bo_bc")
            bsel_sb = const.tile([P, 2], F32, tag="bsel")
            nc.sync.dma_start(bsel_sb[:], bsel_d)

            # persistent activations: q/k [p, pair, kslab, tok] with kslab 1
            # zeroed (DoubleRow zero-pad), v [p, tc, head, 80] fp8
            KSL = 2 if FP8_SC else 1
            qT = persist.tile([P, MC, KSL, S], DQK, tag="qT")
            kT = persist.tile([P, MC, KSL, S], DQK, tag="kT")
            v_sb = persist.tile([P, NTC, NH_LOC, VP], DP, tag="v")
            wo_sb = persist.tile([P, DC, D], BF16, tag="wo")

            if FP8_SC:
                nc.gpsimd.memset(qT[:, :, 1, :], 0.0)
                nc.gpsimd.memset(kT[:, :, 1, :], 0.0)
            # ones column of v (denominator trick): col 64 of each head block
            nc.gpsimd.memset(v_sb[:, :, :, HD : HD + 1], 1.0)

            a2a_in = [
                dram.tile([NC * HD, TB], BF16, name=f"a2ain{h}", tag=f"a2ain{h}")
                for h in range(NH_LOC)
            ]
            a2a_out = [
                dram.tile([NC * HD, TB], BF16, name=f"a2aout{h}", tag=f"a2aout{h}")
                for h in range(NH_LOC)
            ]

            with (
                tc.tile_pool(name="xw", bufs=1) as xw,
            ):
                wq_sb = xw.tile([P, DC, G], DX, tag="wq")
                wk_sb = xw.tile([P, DC, G], DX, tag="wk")
                wv_sb = xw.tile([P, DC, G], DX, tag="wv")
                # bf16 block-0 path: loaded first since QKV(0) starts the
                # whole pipeline
                wq0_sb = xw.tile([P, DC, G], BF16, tag="wq0")
                wk0_sb = xw.tile([P, DC, G], BF16, tag="wk0")
                wv0_sb = xw.tile([P, DC, G], BF16, tag="wv0")
                x0_sb = xw.tile([P, DC, TB], BF16, tag="x0")
                # per-dc-chunk DMAs: the first q matmul chain only waits for
                # the dc slices it has consumed so far, not the whole tile
                wq0_r = wq0_d.rearrange("(dc p) m -> p dc m", p=P)
                x0_r = xt0_d.rearrange("(dc p) t -> p dc t", p=P)
                for dc in range(DC):
                    nc.sync.dma_start(wq0_sb[:, dc, :], wq0_r[:, dc, :])
                    nc.sync.dma_start(x0_sb[:, dc, :], x0_r[:, dc, :])
                nc.sync.dma_start(
                    wk0_sb[:], wk0_d.rearrange("(dc p) m -> p dc m", p=P)
                )
                nc.sync.dma_start(
                    wv0_sb[:], wv0_d.rearrange("(dc p) m -> p dc m", p=P)
                )
                nc.sync.dma_start(wq_sb[:], wq_d.rearrange("(dc p) m -> p dc m", p=P))

                xTt = [
                    xw.tile([P, DC, TB], DX, tag=f"xT{g}", name=f"xT{g}")
                    for g in range(1, NQT)
                ]
                xTg = [None] + [t[:] for t in xTt]
                xt_r = xt_d.rearrange("(dc p) t -> p dc t", p=P)

                for g in range(1, NQT):
                    nc.sync.dma_start(
                        xTg[g], xt_r[:, :, g * TB : (g + 1) * TB]
                    )
                    if g == 1:
                        nc.sync.dma_start(
                            wk_sb[:], wk_d.rearrange("(dc p) m -> p dc m", p=P)
                        )
                        nc.sync.dma_start(
                            wv_sb[:], wv_d.rearrange("(dc p) m -> p dc m", p=P)
                        )

                def emit_qkv(g):
                    if FP8_QKV and g > 0:
                        wq_g, wk_g, wv_g, xg = wq_sb, wk_sb, wv_sb, xTg[g]
                        sc, use_dr = 1.0 / WSC, True
                    else:
                        wq_g, wk_g, wv_g, xg = wq0_sb, wk0_sb, wv0_sb, x0_sb[:]
                        sc, use_dr = 1.0, False
                    for w_sb, b_sb, out_t in ((wq_g, bq_sb, qT), (wk_g, bk_sb, kT)):
                        for mc_i in range(MC):
                            pj = ps_mm.tile(
                                [P, 2, 512], F32, tag="mm", name="pj"
                            )[:, 0, :]
                            if use_dr:
                                for i, dc in enumerate(range(0, DC, 2)):
                                    nc.tensor.matmul(
                                        pj[:],
                                        w_sb[:, dc : dc + 2, mc_i * P : (mc_i + 1) * P],
                                        xg[:, dc : dc + 2, :],
                                        start=(i == 0),
                                        stop=(dc == DC - 2),
                                        perf_mode=DR,
                                    )
                            else:
                                for dc in range(DC):
                                    nc.tensor.matmul(
                                        pj[:],
                                        w_sb[:, dc, mc_i * P : (mc_i + 1) * P],
                                        xg[:, dc, :],
                                        start=(dc == 0),
                                        stop=(dc == DC - 1),
                                    )
                            nc.scalar.activation(
                                out_t[:, mc_i, 0, g * TB : (g + 1) * TB],
                                pj[:],
                                Ident,
                                bias=b_sb[:, mc_i : mc_i + 1],
                                scale=sc,
                            )
                    for ti in range(4):
                        tc_i = 4 * g + ti
                        pv = ps_mm.tile(
                            [P, 2, 512], F32, tag="mm", name="pv"
                        )[:, 0, :]
                        if use_dr:
                            for i, dc in enumerate(range(0, DC, 2)):
                                nc.tensor.matmul(
                                    pv[:, 0:G],
                                    xg[:, dc : dc + 2, ti * P : (ti + 1) * P],
                                    wv_g[:, dc : dc + 2, :],
                                    start=(i == 0),
                                    stop=(dc == DC - 2),
                                    perf_mode=DR,
                                )
                        else:
                            for dc in range(DC):
                                nc.tensor.matmul(
                                    pv[:, 0:G],
                                    xg[:, dc, ti * P : (ti + 1) * P],
                                    wv_g[:, dc, :],
                                    start=(dc == 0),
                                    stop=(dc == DC - 1),
                                )
                        # v = pv*sc + bv into padded head blocks
                        nc.vector.scalar_tensor_tensor(
                            v_sb[:, tc_i, :, 0:HD],
                            pv[:, 0:G].rearrange("p (h c) -> p h c", c=HD),
                            sc,
                            bv_bc[:].rearrange("p (h c) -> p h c", c=HD),
                            MULT,
                            ADD,
                        )

                # ------ attention, emission-interleaved with QKV blocks ------
                nc.sync.dma_start(
                    wo_sb[:], wo_d.rearrange("(dc p) n -> p dc n", p=P)
                )
                nc.sync.dma_start(bo_row[:], bo_d)
                nc.gpsimd.partition_broadcast(bo_bc[:], bo_row[:])
                work = tc.alloc_tile_pool(name="att", bufs=1)
                pTp = tc.alloc_tile_pool(name="pTp", bufs=8)
                smallp = tc.alloc_tile_pool(name="smallp", bufs=3)
                ctxn = [
                    [
                        work.tile(
                            [HD, TB], BF16, tag=f"ctxn{h}_{q}", name=f"ctxn{h}_{q}"
                        )
                        for q in range(NQT)
                    ]
                    for h in range(NH_LOC)
                ]
                # reciprocal of denominator lives at row 64; other rows stay 0
                # so the broadcast matmul (ones at row 64) reads no garbage.
                # double-buffered by qt-unit parity: the deferred norm of unit
                # i reads its rden while unit i+1's reciprocal writes the other
                rdenX = [
                    work.tile([P, 512], F32, tag=f"rdenX{i}", name=f"rdenX{i}")
                    for i in range(4)
                ]
                for i in range(4):
                    nc.vector.tensor_copy(rdenX[i][:], zeros_f[:])
                c_ps_of = {}
                unit_ctr = [0]

                def emit_scores(pair, ch):
                    qt, k0, k1 = ch
                    p_tiles = {}
                    for kc in range(k0, k1):
                        j = kc - 4 * qt
                        coff = max(0, j) * P
                        if kc % 2 == 0:
                            pT = pTp.tile([P, 2, 2, TB], DP, tag="pT")
                            p_tiles[kc] = pT
                        else:
                            pT = p_tiles[kc - 1]
                        s_ps = ps_mm.tile([P, 2, 512], F32, tag="mm")
                        for h01 in range(2):
                            pb = h01 * HD
                            if FP8_SC:
                                nc.tensor.matmul(
                                    s_ps[:, h01, coff:512],
                                    kT[pb : pb + HD, pair, :, kc * P : (kc + 1) * P],
                                    qT[
                                        pb : pb + HD,
                                        pair,
                                        :,
                                        qt * TB + coff : (qt + 1) * TB,
                                    ],
                                    start=True,
                                    stop=True,
                                    perf_mode=DR,
                                )
                            else:
                                nc.tensor.matmul(
                                    s_ps[:, h01, coff:512],
                                    kT[pb : pb + HD, pair, 0, kc * P : (kc + 1) * P],
                                    qT[
                                        pb : pb + HD,
                                        pair,
                                        0,
                                        qt * TB + coff : (qt + 1) * TB,
                                    ],
                                    start=True,
                                    stop=True,
                                )
                        nc.scalar.activation(
                            pT[:, kc % 2, :, coff:512],
                            s_ps[:, :, coff:512],
                            Exp,
                            scale=0.125,
                        )
                        if j >= 0:
                            nc.vector.tensor_tensor(
                                pT[:, kc % 2, :, coff : coff + P],
                                pT[:, kc % 2, :, coff : coff + P],
                                tri_p[:],
                                MULT,
                            )
                    return p_tiles

                def emit_ctx(pair, ch, p_tiles, tail=False):
                    qt, k0, k1 = ch
                    nkc = 4 * qt + 4
                    if k0 == 0:
                        c_ps_of[pair, qt] = [
                            ps_ctx.tile([P, 512], F32, tag="ctx", name=f"cps{h01}")
                            for h01 in range(2)
                        ]
                    c_ps = c_ps_of[pair, qt]
                    final = k1 == nkc
                    if final:
                        rdens = [
                            rdenX[2 * (unit_ctr[0] % 2) + h01]
                            for h01 in range(2)
                        ]
                        unit_ctr[0] += 1
                    # head-serial: head h01's ctx and reciprocal go out
                    # before head h01+1's ctx, so the 3.3us DVE reciprocal
                    # overlaps the other head's ctx matmuls; the rest of the
                    # normalize (emit_norm) is deferred one push unit so the
                    # broadcast matmul never stalls the in-order PE queue
                    for h01 in range(2):
                        h = 2 * pair + h01
                        for kc in range(k0, k1):
                            j = kc - 4 * qt
                            pT = p_tiles[kc - kc % 2]
                            coff = max(0, j) * P
                            nc.tensor.matmul(
                                c_ps[h01][0 : HD + 1, coff:512],
                                v_sb[:, kc, h, 0 : HD + 1],
                                pT[:, kc % 2, h01, coff:512],
                                start=(kc == 0),
                                stop=(kc == nkc - 1),
                            )
                        if not final:
                            continue
                        rX = rdens[h01]
                        if tail:
                            # final flush gates the last collective triggers
                            # and ACT is idle there: reciprocal as exp(-ln)
                            # on ACT, off the DVE queue
                            lnd = smallp.tile([P, 512], F32, tag="bb")
                            nc.scalar.activation(
                                lnd[64:65, :],
                                c_ps[h01][64:65, :],
                                mybir.ActivationFunctionType.Ln,
                            )
                            nc.scalar.activation(
                                rX[64:65, :], lnd[64:65, :], Exp, scale=-1.0
                            )
                        else:
                            with nc.allow_low_precision(reason="den recip"):
                                nc.vector.reciprocal(
                                    rX[64:65, :], c_ps[h01][64:65, :]
                                )
                    if final:
                        norm_pend.append((pair, qt, rdens))

                def emit_norm(pair, qt, rdens):
                    # normalize: divide by the ones-row sums, broadcast
                    # across partitions via a row-64 matmul; then the A2A
                    # sends (destination block qt, duplicated across batch
                    # halves) and on qt3 head h's collective
                    c_ps = c_ps_of.pop((pair, qt))
                    for h01 in range(2):
                        h = 2 * pair + h01
                        b_ps = ps_mm.tile([P, 2, 512], F32, tag="mm", name="bps")[
                            :, 0, :
                        ]
                        nc.tensor.matmul(
                            b_ps[0:HD, :],
                            zrow_f[:, 0:HD],
                            rdens[h01][:],
                            start=True,
                            stop=True,
                        )
                        bb = smallp.tile([HD, 512], F32, tag="bb")
                        nc.scalar.copy(bb[:], b_ps[0:HD, :])
                        nc.vector.tensor_tensor(
                            ctxn[h][qt][:, :],
                            c_ps[h01][0:HD, :],
                            bb[:],
                            MULT,
                        )
                        for sh in (qt, qt + 4):
                            nc.sync.dma_start(
                                a2a_in[h][sh * HD : (sh + 1) * HD, :],
                                ctxn[h][qt][:, :],
                            )
                        if qt == NQT - 1 and coll:
                            nc.gpsimd.collective_compute(
                                "AllToAll",
                                mybir.AluOpType.bypass,
                                ins=[a2a_in[h].opt()],
                                outs=[a2a_out[h].opt()],
                                replica_groups=[list(range(NC))],
                            )

                def qt_chunks(qt):
                    return [
                        (qt, kcb, min(kcb + 8, 4 * qt + 4))
                        for kcb in range(0, 4 * qt + 4, 8)
                    ]

                # software pipeline: emit scores(i+1) before ctx(i) so the PE
                # queue never blocks the ACT exp stream behind ctx matmuls;
                # norms run one more push behind so reciprocals are long done;
                # pair-0 attention interleaves with the QKV q-block loop
                pend = []
                norm_pend = []

                def push(pair, ch):
                    tiles = emit_scores(pair, ch)
                    # only norms queued BEFORE this push (one-unit deferral)
                    ready_norms = norm_pend[:]
                    del norm_pend[:]
                    if pend:
                        emit_ctx(*pend.pop())
                    for n in ready_norms:
                        emit_norm(*n)
                    pend.append((pair, ch, tiles))

                # pair-1 staggered one q-block behind pair-0: its exp
                # bursts fill the ACT gaps left by each QKV block
                for g in range(NQT):
                    emit_qkv(g)
                    for ch in qt_chunks(g):
                        push(0, ch)
                    if g == NQT - 1:
                        # force-flush pair-0 qt3's ctx+norm before pair-1
                        # qt2 scores so A2A(h0)/A2A(h1) fire ~10us earlier
                        emit_ctx(*pend.pop())
                    if g >= 1:
                        for ch in qt_chunks(g - 1):
                            push(1, ch)
                emit_ctx(*pend.pop())
                for ch in qt_chunks(NQT - 1):
                    push(1, ch)
                _fp, _fc, _ft = pend.pop()
                emit_ctx(_fp, _fc, _ft, tail=True)
                for n in norm_pend:
                    emit_norm(*n)
                del norm_pend[:]

                smallp.release()
                pTp.release()
                work.release()

            # ---------- receive + output projection ----------
            # per-head receive: a2a_out[h] rows = 8 senders x 64 dims of
            # local head h for my token block; jj blocks of 128 rows pair
            # senders (2jj, 2jj+1), jj 0-1 batch 0, jj 2-3 batch 1
            outp = tc.alloc_tile_pool(name="outp", bufs=1)
            gsrc = a2a_out if coll else a2a_in
            cx = outp.tile([P, NH_LOC, 4, TB], BF16, tag="cx")
            tmp = outp.tile([P, 2, TB], BF16, tag="seltmp")
            # ctxf[p, h, jj, t]: wo_sb chunk h*2+jj (Wo rows host-permuted)
            ctxf = outp.tile([P, NH_LOC, 2, TB], BF16, tag="ctxf")

            def recv_head(h):
                # readback + batch select (ctxf = lo*bsel[0] + hi*bsel[1])
                # per column half: the first po chains start on half the data
                for hh in range(2):
                    hc = slice(hh * 256, (hh + 1) * 256)
                    nc.sync.dma_start(
                        cx[:, h, :, hc],
                        gsrc[h][:, hc].rearrange("(j q) t -> q j t", q=P),
                    )
                    nc.vector.tensor_scalar(
                        tmp[:, :, hc], cx[:, h, 0:2, hc], bsel_sb[:, 0:1],
                        None, MULT,
                    )
                    nc.vector.scalar_tensor_tensor(
                        ctxf[:, h, :, hc], cx[:, h, 2:4, hc],
                        bsel_sb[:, 1:2], tmp[:, :, hc], MULT, ADD,
                    )

            with tc.tile_pool(name="out_pool", bufs=3) as out_pool:
                o_parts = [
                    outp.tile([P, 512], F32, tag=f"opart{u}", name=f"opart{u}")
                    for u in range(8)
                ]
                # heads 0-1 partials: overlap with A2As of heads 2-3
                recv_head(0)
                recv_head(1)
                for u in range(8):
                    tc_i, nt = u // 2, u % 2
                    po = ps_mm.tile([P, 2, 512], F32, tag="mm")
                    for i, (h, jj) in enumerate(
                        [(0, 0), (0, 1), (1, 0), (1, 1)]
                    ):
                        nc.tensor.matmul(
                            po[:, 0, :],
                            ctxf[:, h, jj, tc_i * P : (tc_i + 1) * P],
                            wo_sb[:, h * 2 + jj, nt * 512 : (nt + 1) * 512],
                            start=(i == 0),
                            stop=(i == 3),
                        )
                    nc.vector.tensor_tensor(
                        o_parts[u][:],
                        po[:, 0, :],
                        bo_bc[:, nt * 512 : (nt + 1) * 512],
                        ADD,
                    )
                # head-2 partials accumulate into o_parts under A2A of head 3
                recv_head(2)
                for u in range(8):
                    tc_i, nt = u // 2, u % 2
                    po = ps_mm.tile([P, 2, 512], F32, tag="mm")
                    for jj in range(2):
                        nc.tensor.matmul(
                            po[:, 0, :],
                            ctxf[:, 2, jj, tc_i * P : (tc_i + 1) * P],
                            wo_sb[:, 4 + jj, nt * 512 : (nt + 1) * 512],
                            start=(jj == 0),
                            stop=(jj == 1),
                        )
                    nc.vector.tensor_tensor(
                        o_parts[u][:], po[:, 0, :], o_parts[u][:], ADD
                    )
                # keep-warm: a few dummy matmuls bridging to the last A2A so
                # the PE doesn't re-throttle (>3us idle drops HAM to half
                # clock) before the head-3 output projection
                warm = ps_mm.tile([P, 2, 512], F32, tag="mm", name="warm")
                for wj in range(12):
                    nc.tensor.matmul(
                        warm[:, 0, :],
                        ctxf[:, 0, wj % 2, 0:P],
                        wo_sb[:, 0, 0:512],
                        start=True,
                        stop=True,
                    )
                recv_head(3)
                for u in range(8):
                    tc_i, nt = u // 2, u % 2
                    po = ps_mm.tile([P, 2, 512], F32, tag="mm")
                    for jj in range(2):
                        nc.tensor.matmul(
                            po[:, 0, :],
                            ctxf[:, 3, jj, tc_i * P : (tc_i + 1) * P],
                            wo_sb[:, 6 + jj, nt * 512 : (nt + 1) * 512],
                            start=(jj == 0),
                            stop=(jj == 1),
                        )
                    o_sb = out_pool.tile([P, 512], F32, tag="osb")
                    nc.vector.tensor_tensor(
                        o_sb[:], po[:, 0, :], o_parts[u][:], ADD
                    )
                    nc.sync.dma_start(
                        y_d[
                            tc_i * P : (tc_i + 1) * P,
                            nt * 512 : (nt + 1) * 512,
                        ],
                        o_sb[:],
                    )

            outp.release()

    nc.compile()
    return nc


_NC_CACHE = {}


def _get_nc():
    if "nc" not in _NC_CACHE:
        _NC_CACHE["nc"] = build()
    return _NC_CACHE["nc"]


def _make_in_maps(x, Wq, bq, Wk, bk, Wv, bv, Wo, bo):
    x = np.asarray(x, np.float32)
    Wq, Wk, Wv, Wo = (np.asarray(a, np.float32) for a in (Wq, Wk, Wv, Wo))
    bq, bk, bv, bo = (np.asarray(a, np.float32) for a in (bq, bk, bv, bo))
    np_dx = NP_F8 if FP8_QKV else NP_BF16
    # permute Wo rows to the per-head receive order: sbuf chunk c = h*2+jj,
    # partition p holds sender group g' = 2*jj + p//64's head h, dim p%64
    # (global head g'*4 + h)
    perm = np.empty(D, np.int64)
    for ci in range(DC):
        h, jj = ci // 2, ci % 2
        for par in range(2):
            H = (2 * jj + par) * 4 + h
            base = ci * P + par * HD
            perm[base : base + HD] = np.arange(H * HD, (H + 1) * HD)
    wo_b = np.ascontiguousarray(Wo[perm]).astype(NP_BF16)
    in_maps = []
    for c in range(NC):
        b, g = c // 4, c % 4
        sl = slice(g * G, (g + 1) * G)
        bsel = np.tile(
            np.array([1.0 - b, float(b)], np.float32).reshape(1, 2), (P, 1)
        )
        in_maps.append(
            {
                "xt": np.ascontiguousarray(x[b].T).astype(np_dx),
                "wq": np.ascontiguousarray(Wq[:, sl] * WSC).astype(np_dx),
                "wk": np.ascontiguousarray(Wk[:, sl] * WSC).astype(np_dx),
                "wv": np.ascontiguousarray(Wv[:, sl] * WSC).astype(np_dx),
                "xt0": np.ascontiguousarray(x[b].T[:, :TB]).astype(NP_BF16),
                "wq0": np.ascontiguousarray(Wq[:, sl]).astype(NP_BF16),
                "wk0": np.ascontiguousarray(Wk[:, sl]).astype(NP_BF16),
                "wv0": np.ascontiguousarray(Wv[:, sl]).astype(NP_BF16),
                "bq": np.ascontiguousarray(bq[sl].reshape(MC, P).T),
                "bk": np.ascontiguousarray(bk[sl].reshape(MC, P).T),
                "bv": np.ascontiguousarray(bv[sl].reshape(1, G)),
                "wo": wo_b,
                "bo": np.ascontiguousarray(bo.reshape(1, D)),
                "bsel": np.ascontiguousarray(bsel),
            }
        )
    return in_maps


def run(inputs, trace=False, tmpdir=None):
    """Run on 8 cores; returns (output [2,2048,1024], BassKernelResults)."""
    if trace:
        _install_ntff_hook()
    nc = _get_nc()
    in_maps = _make_in_maps(**inputs)
    res = bass_utils.run_bass_kernel_spmd(
        nc, in_maps, core_ids=list(range(NC)), trace=trace, tmpdir=tmpdir
    )
    out = np.empty((B, S, D), np.float32)
    for c in range(NC):
        b, g = c // 4, c % 4
        out[b, g * TB : (g + 1) * TB, :] = res.results[c]["y"]
    return out, res


def kernel(**inputs) -> np.ndarray:
    out, _ = run(inputs, trace=False)
    return out



# revision 42
# speedup vs baseline: 1.2158x; 1.0882x over previous
"""GPT self-attention layer (B=2, S=2048, D=1024, H=16, hd=64) on 8 TRN2 cores.

Sharding: data-parallel over batch (2) x tensor-parallel over heads (4 groups
of 4 heads). Core c handles batch b=c//4, head group g=c%4.

Per-core pipeline (bf16 attention, fp8-DoubleRow projections):
  1. QKV projections per 512-token block: fp8e4m3 x/W inputs with DoubleRow
     matmuls (4x PE rate) for blocks 1-3; block 0 takes a bf16 side path
     (xt0/wq0/wk0/wv0 inputs) because the ~3.5% fp8 x*W value error is
     amplified by the short causal softmax rows that all live there. fp8
     weights are host-prescaled by 64 (subnormal dodge) and descaled for
     free in the ACT bias op / DVE V-store. q/k/v stored bf16; V carries an
     interleaved ones column so the softmax denominator accumulates in ctx
     psum row 64 for free.
  2. Attention per head pair, scoresT [k-part, q-free], exp on ACT into
     bf16 pT (kc-pair tiles), causal diag masked by a DVE multiply, ctx
     accumulated per kc head-serial. 1/den via DVE reciprocal (row 64),
     broadcast across partitions by an f32r matmul against a
     ones-at-row-64 column (1 cycle/row vs fp32's 4), DVE-staged
     normalize -> bf16.
  3. Emission is software-pipelined three ways: scores(chunk i+1) go out
     before ctx(chunk i) so ctx never blocks the ACT exp stream; each
     unit's normalize (reciprocal included) is deferred one further push
     so the 3.3us DVE reciprocal sits behind the next chunk's diag masks
     in the DVE FIFO and its broadcast matmul never stalls the in-order
     PE queue; pair-0 rides the QKV block loop so its qt3 completes right
     after QKV(3).
  4. Three bf16 AllToAlls, shards duplicated across batch halves: two
     per-head 0.5MB ones for heads 0/1, fired right after QKV(3) and
     hidden under all of pair-1's attention, and one merged 1MB one for
     heads 2+3 at the end (a single transfer beats two serialized ones
     there). gpsimd holds only collectives, so the waiting triggers stall
     nothing. Receiver picks its batch half with a DVE mask-select driven
     by the bsel input (no gather).
  5. out = ctxT.T @ Wo + bo in bf16, Wo host-permuted to the receive
     order (per-head chunks 0-3, paired chunks 4-7): heads 0/1 partial
     sums plus keep-warm matmuls (HAM re-throttles after >3us PE idle)
     run under the pair-1 AllToAll; its receive+select+projection is
     pipelined per 128-col token quarter; DMA to y [512,1024] fp32.
     Startup block-0 loads are split per dc chunk so the first q matmul
     starts as soon as its first slices land.
"""

import contextlib
import ctypes
import sys
import types

sys.path.insert(0, "/opt/trn_rl_repo")

import numpy as np
import ml_dtypes

import concourse.bass as bass
import concourse.mybir as mybir
import concourse.tile as tile
from concourse import bacc
from concourse import bass_utils

P = 128
B, S, D = 2, 2048, 1024
NH_LOC = 4          # heads per core
HD = 64             # head dim
G = NH_LOC * HD     # local head dims = 256
MC = G // P         # m-chunks of local dims = 2
DC = D // P         # d-chunks = 8
TB = 512            # token block (output tokens per core, q-tile width)
NQT = S // TB       # q-tiles = 4
NTC = S // P        # token chunks = 16
NC = 8
VP = 80             # per-head v block stride (64 data + 1 ones + 15 pad)

F32 = mybir.dt.float32
F32R = mybir.dt.float32r
BF16 = mybir.dt.bfloat16
F8 = mybir.dt.float8e4
Exp = mybir.ActivationFunctionType.Exp
Ident = mybir.ActivationFunctionType.Identity
MULT = mybir.AluOpType.mult
ADD = mybir.AluOpType.add
DR = mybir.MatmulPerfMode.DoubleRow

# per-stage dtype switches (bf16 fallback for numerics bisection).
# FP8_QKV runs the q/k/v projections in fp8 DoubleRow (4x PE) for token
# blocks 1-3; block 0 always takes a bf16 side path because the fp8 x*W
# value error (~3.5%) is amplified by the short causal softmax rows that
# all live in the first block.
FP8_QKV = True
FP8_SC = True
FP8_CTX = False
WSC = 64.0 if FP8_QKV else 1.0  # host weight prescale

DX = F8 if FP8_QKV else BF16    # xt, wq/wk/wv
DQK = F8 if FP8_SC else BF16    # stored q / k
DP = F8 if FP8_CTX else BF16    # pT probs and v

NP_BF16 = np.dtype(ml_dtypes.bfloat16)
NP_F8 = np.dtype(ml_dtypes.float8_e4m3)


def _install_ntff_hook():
    """Make trace=True work under axon: inject antenv.axon_hooks backed by
    ctypes calls into libaxon_pjrt.so (mirrors trn_agent_boot logic)."""
    if "antenv.axon_hooks" in sys.modules:
        return
    holder = {}
    mod = types.ModuleType("antenv.axon_hooks")
    mod.set_axon_ntff_profile_hook = lambda h: holder.update(h=h)
    mod.get_axon_ntff_profile_hook = lambda: holder.get("h")
    sys.modules["antenv.axon_hooks"] = mod
    try:
        lib = ctypes.CDLL("/opt/axon/libaxon_pjrt.so")
        if not hasattr(lib, "axon_start_nrt_profile"):
            return
    except OSError:
        return
    lib.axon_start_nrt_profile.argtypes = [
        ctypes.POINTER(ctypes.c_int64),
        ctypes.c_size_t,
    ]
    lib.axon_start_nrt_profile.restype = ctypes.c_int64
    lib.axon_stop_nrt_profile.argtypes = [ctypes.c_char_p]
    lib.axon_stop_nrt_profile.restype = ctypes.c_int64

    @contextlib.contextmanager
    def _hook(output_dir, device_ids):
        import jax

        jax.devices()
        if device_ids:
            ids = (ctypes.c_int64 * len(device_ids))(*device_ids)
            rc = lib.axon_start_nrt_profile(ids, len(device_ids))
        else:
            rc = lib.axon_start_nrt_profile(None, 0)
        if rc != 0:
            raise RuntimeError(f"axon_start_nrt_profile rc={rc}")
        try:
            yield
        finally:
            n = lib.axon_stop_nrt_profile(str(output_dir).encode())
            print(f"profile: {n} ntff file(s) written to {output_dir}")

    holder["h"] = _hook


def build(coll=True):
    nc = bacc.Bacc("TRN2", target_bir_lowering=False, debug=False, num_devices=NC)

    xt_d = nc.dram_tensor("xt", [D, S], DX, kind="ExternalInput").ap()
    wq_d = nc.dram_tensor("wq", [D, G], DX, kind="ExternalInput").ap()
    wk_d = nc.dram_tensor("wk", [D, G], DX, kind="ExternalInput").ap()
    wv_d = nc.dram_tensor("wv", [D, G], DX, kind="ExternalInput").ap()
    xt0_d = nc.dram_tensor("xt0", [D, TB], BF16, kind="ExternalInput").ap()
    wq0_d = nc.dram_tensor("wq0", [D, G], BF16, kind="ExternalInput").ap()
    wk0_d = nc.dram_tensor("wk0", [D, G], BF16, kind="ExternalInput").ap()
    wv0_d = nc.dram_tensor("wv0", [D, G], BF16, kind="ExternalInput").ap()
    bq_d = nc.dram_tensor("bq", [P, MC], F32, kind="ExternalInput").ap()
    bk_d = nc.dram_tensor("bk", [P, MC], F32, kind="ExternalInput").ap()
    bv_d = nc.dram_tensor("bv", [1, G], F32, kind="ExternalInput").ap()
    wo_d = nc.dram_tensor("wo", [D, D], BF16, kind="ExternalInput").ap()
    bo_d = nc.dram_tensor("bo", [1, D], F32, kind="ExternalInput").ap()
    bsel_d = nc.dram_tensor("bsel", [P, 2], F32, kind="ExternalInput").ap()
    y_d = nc.dram_tensor("y", [TB, D], F32, kind="ExternalOutput").ap()

    with tile.TileContext(nc) as tc:
        with (
            tc.tile_pool(name="const", bufs=1) as const,
            tc.tile_pool(name="dram", bufs=1, space="DRAM") as dram,
            tc.tile_pool(name="ps_mm", bufs=2, space="PSUM") as ps_mm,
            tc.tile_pool(name="ps_ctx", bufs=4, space="PSUM") as ps_ctx,
            tc.tile_pool(name="persist", bufs=1) as persist,
        ):
            # ---------------- constants ----------------
            ones_f = const.tile([P, 1], F32, tag="ones_f")
            nc.vector.memset(ones_f[:], 1.0)
            # trimask[k, u] = 1 if k <= u else 0 (keep where u - k >= 0)
            tri_f = const.tile([P, P], F32, tag="tri_f")
            nc.gpsimd.memset(tri_f[:], 1.0)
            nc.gpsimd.affine_select(
                out=tri_f[:],
                in_=tri_f[:],
                compare_op=mybir.AluOpType.is_ge,
                fill=0.0,
                base=0,
                pattern=[[1, P]],
                channel_multiplier=-1,
            )
            # materialized for both heads so the gpsimd mask multiply uses a
            # plain strided AP (no broadcast)
            tri_p = const.tile([P, 2, P], DP, tag="tri_p")
            nc.vector.tensor_copy(
                tri_p[:], tri_f[:, None, :].to_broadcast((P, 2, P))
            )
            # ones at row 64 only: broadcast-den matmul weights
            zrow_f = const.tile([P, HD], F32, tag="zrow_f")
            nc.vector.memset(zrow_f[:], 0.0)
            nc.vector.memset(zrow_f[64:65, :], 1.0)
            onescol_r = const.tile([P, HD], F32R, tag="onescol_r")
            nc.vector.tensor_copy(onescol_r[:], zrow_f[:])
            zeros_f = const.tile([P, 512], F32, tag="zeros_f")
            nc.vector.memset(zeros_f[:], 0.0)

            bq_sb = const.tile([P, MC], F32, tag="bq")
            bk_sb = const.tile([P, MC], F32, tag="bk")
            nc.sync.dma_start(bq_sb[:], bq_d)
            nc.sync.dma_start(bk_sb[:], bk_d)
            bv_row = const.tile([1, G], F32, tag="bv_row")
            nc.sync.dma_start(bv_row[:], bv_d)
            bv_bc = const.tile([P, G], F32, tag="bv_bc")
            nc.gpsimd.partition_broadcast(bv_bc[:], bv_row[:])
            bo_row = const.tile([1, D], F32, tag="bo_row")
            bo_bc = const.tile([P, D], F32, tag="bo_bc")
            bsel_sb = const.tile([P, 2], F32, tag="bsel")
            nc.sync.dma_start(bsel_sb[:], bsel_d)

            # persistent activations: q/k [p, pair, kslab, tok] with kslab 1
            # zeroed (DoubleRow zero-pad), v [p, tc, head, 80] fp8
            KSL = 2 if FP8_SC else 1
            qT = persist.tile([P, MC, KSL, S], DQK, tag="qT")
            kT = persist.tile([P, MC, KSL, S], DQK, tag="kT")
            v_sb = persist.tile([P, NTC, NH_LOC, VP], DP, tag="v")
            wo_sb = persist.tile([P, DC, D], BF16, tag="wo")

            if FP8_SC:
                nc.gpsimd.memset(qT[:, :, 1, :], 0.0)
                nc.gpsimd.memset(kT[:, :, 1, :], 0.0)
            # ones column of v (denominator trick): col 64 of each head block
            nc.gpsimd.memset(v_sb[:, :, :, HD : HD + 1], 1.0)

            # heads 0/1 get per-head 0.5MB buffers (their collectives hide
            # under pair-1 attention); heads 2/3 share one 1MB buffer so the
            # end-gated exchange is a single transfer, not two serialized
            a2a_in = [
                dram.tile([NC * HD, TB], BF16, name="a2ain0", tag="a2ain0"),
                dram.tile([NC * HD, TB], BF16, name="a2ain1", tag="a2ain1"),
                dram.tile([NC * P, TB], BF16, name="a2ainp1", tag="a2ainp1"),
            ]
            a2a_out = [
                dram.tile([NC * HD, TB], BF16, name="a2aout0", tag="a2aout0"),
                dram.tile([NC * HD, TB], BF16, name="a2aout1", tag="a2aout1"),
                dram.tile([NC * P, TB], BF16, name="a2aoutp1", tag="a2aoutp1"),
            ]

            with (
                tc.tile_pool(name="xw", bufs=1) as xw,
            ):
                wq_sb = xw.tile([P, DC, G], DX, tag="wq")
                wk_sb = xw.tile([P, DC, G], DX, tag="wk")
                wv_sb = xw.tile([P, DC, G], DX, tag="wv")
                # bf16 block-0 path: loaded first since QKV(0) starts the
                # whole pipeline
                wq0_sb = xw.tile([P, DC, G], BF16, tag="wq0")
                wk0_sb = xw.tile([P, DC, G], BF16, tag="wk0")
                wv0_sb = xw.tile([P, DC, G], BF16, tag="wv0")
                x0_sb = xw.tile([P, DC, TB], BF16, tag="x0")
                # per-dc-chunk DMAs: the first q matmul chain only waits for
                # the dc slices it has consumed so far, not the whole tile
                wq0_r = wq0_d.rearrange("(dc p) m -> p dc m", p=P)
                x0_r = xt0_d.rearrange("(dc p) t -> p dc t", p=P)
                for dc in range(DC):
                    nc.sync.dma_start(wq0_sb[:, dc, :], wq0_r[:, dc, :])
                    nc.sync.dma_start(x0_sb[:, dc, :], x0_r[:, dc, :])
                nc.sync.dma_start(
                    wk0_sb[:], wk0_d.rearrange("(dc p) m -> p dc m", p=P)
                )
                nc.sync.dma_start(
                    wv0_sb[:], wv0_d.rearrange("(dc p) m -> p dc m", p=P)
                )
                nc.sync.dma_start(wq_sb[:], wq_d.rearrange("(dc p) m -> p dc m", p=P))

                xTt = [
                    xw.tile([P, DC, TB], DX, tag=f"xT{g}", name=f"xT{g}")
                    for g in range(1, NQT)
                ]
                xTg = [None] + [t[:] for t in xTt]
                xt_r = xt_d.rearrange("(dc p) t -> p dc t", p=P)

                for g in range(1, NQT):
                    nc.sync.dma_start(
                        xTg[g], xt_r[:, :, g * TB : (g + 1) * TB]
                    )
                    if g == 1:
                        nc.sync.dma_start(
                            wk_sb[:], wk_d.rearrange("(dc p) m -> p dc m", p=P)
                        )
                        nc.sync.dma_start(
                            wv_sb[:], wv_d.rearrange("(dc p) m -> p dc m", p=P)
                        )

                def emit_qkv(g):
                    if FP8_QKV and g > 0:
                        wq_g, wk_g, wv_g, xg = wq_sb, wk_sb, wv_sb, xTg[g]
                        sc, use_dr = 1.0 / WSC, True
                    else:
                        wq_g, wk_g, wv_g, xg = wq0_sb, wk0_sb, wv0_sb, x0_sb[:]
                        sc, use_dr = 1.0, False
                    for w_sb, b_sb, out_t in ((wq_g, bq_sb, qT), (wk_g, bk_sb, kT)):
                        for mc_i in range(MC):
                            pj = ps_mm.tile(
                                [P, 2, 512], F32, tag="mm", name="pj"
                            )[:, 0, :]
                            if use_dr:
                                for i, dc in enumerate(range(0, DC, 2)):
                                    nc.tensor.matmul(
                                        pj[:],
                                        w_sb[:, dc : dc + 2, mc_i * P : (mc_i + 1) * P],
                                        xg[:, dc : dc + 2, :],
                                        start=(i == 0),
                                        stop=(dc == DC - 2),
                                        perf_mode=DR,
                                    )
                            else:
                                for dc in range(DC):
                                    nc.tensor.matmul(
                                        pj[:],
                                        w_sb[:, dc, mc_i * P : (mc_i + 1) * P],
                                        xg[:, dc, :],
                                        start=(dc == 0),
                                        stop=(dc == DC - 1),
                                    )
                            nc.scalar.activation(
                                out_t[:, mc_i, 0, g * TB : (g + 1) * TB],
                                pj[:],
                                Ident,
                                bias=b_sb[:, mc_i : mc_i + 1],
                                scale=sc,
                            )
                    for ti in range(4):
                        tc_i = 4 * g + ti
                        pv = ps_mm.tile(
                            [P, 2, 512], F32, tag="mm", name="pv"
                        )[:, 0, :]
                        if use_dr:
                            for i, dc in enumerate(range(0, DC, 2)):
                                nc.tensor.matmul(
                                    pv[:, 0:G],
                                    xg[:, dc : dc + 2, ti * P : (ti + 1) * P],
                                    wv_g[:, dc : dc + 2, :],
                                    start=(i == 0),
                                    stop=(dc == DC - 2),
                                    perf_mode=DR,
                                )
                        else:
                            for dc in range(DC):
                                nc.tensor.matmul(
                                    pv[:, 0:G],
                                    xg[:, dc, ti * P : (ti + 1) * P],
                                    wv_g[:, dc, :],
                                    start=(dc == 0),
                                    stop=(dc == DC - 1),
                                )
                        # v = pv*sc + bv into padded head blocks
                        nc.vector.scalar_tensor_tensor(
                            v_sb[:, tc_i, :, 0:HD],
                            pv[:, 0:G].rearrange("p (h c) -> p h c", c=HD),
                            sc,
                            bv_bc[:].rearrange("p (h c) -> p h c", c=HD),
                            MULT,
                            ADD,
                        )

                # ------ attention, emission-interleaved with QKV blocks ------
                nc.sync.dma_start(
                    wo_sb[:], wo_d.rearrange("(dc p) n -> p dc n", p=P)
                )
                nc.sync.dma_start(bo_row[:], bo_d)
                nc.gpsimd.partition_broadcast(bo_bc[:], bo_row[:])
                work = tc.alloc_tile_pool(name="att", bufs=1)
                pTp = tc.alloc_tile_pool(name="pTp", bufs=8)
                smallp = tc.alloc_tile_pool(name="smallp", bufs=3)
                ctxn = [
                    [
                        work.tile(
                            [HD, TB], BF16, tag=f"ctxn{h}_{q}", name=f"ctxn{h}_{q}"
                        )
                        for q in range(NQT)
                    ]
                    for h in range(NH_LOC)
                ]
                # reciprocal of denominator lives at row 64; other rows stay 0
                # so the broadcast matmul (ones at row 64) reads no garbage.
                # double-buffered by qt-unit parity: the deferred norm of unit
                # i reads its rden while unit i+1's reciprocal writes the other
                rdenX = [
                    work.tile([P, 512], F32R, tag=f"rdenX{i}", name=f"rdenX{i}")
                    for i in range(4)
                ]
                for i in range(4):
                    nc.vector.tensor_copy(rdenX[i][:], zeros_f[:])
                c_ps_of = {}
                unit_ctr = [0]

                def emit_scores(hs, ch):
                    qt, k0, k1 = ch
                    nh = len(hs)
                    p_tiles = {}
                    for kc in range(k0, k1):
                        j = kc - 4 * qt
                        coff = max(0, j) * P
                        if kc % 2 == 0:
                            pT = pTp.tile([P, 2, nh, TB], DP, tag=f"pT{nh}")
                            p_tiles[kc] = pT
                        else:
                            pT = p_tiles[kc - 1]
                        s_ps = ps_mm.tile([P, 2, 512], F32, tag="mm")
                        for i, h in enumerate(hs):
                            pb = (h % 2) * HD
                            nc.tensor.matmul(
                                s_ps[:, i, coff:512],
                                kT[
                                    pb : pb + HD, h // 2, 0, kc * P : (kc + 1) * P
                                ],
                                qT[
                                    pb : pb + HD,
                                    h // 2,
                                    0,
                                    qt * TB + coff : (qt + 1) * TB,
                                ],
                                start=True,
                                stop=True,
                            )
                        nc.scalar.activation(
                            pT[:, kc % 2, :, coff:512],
                            s_ps[:, 0:nh, coff:512],
                            Exp,
                            scale=0.125,
                        )
                        if j >= 0:
                            nc.vector.tensor_tensor(
                                pT[:, kc % 2, :, coff : coff + P],
                                pT[:, kc % 2, :, coff : coff + P],
                                tri_p[:, 0:nh, :],
                                MULT,
                            )
                    return p_tiles

                def emit_ctx(hs, ch, p_tiles, tail=False):
                    qt, k0, k1 = ch
                    nkc = 4 * qt + 4
                    if k0 == 0:
                        c_ps_of[hs[0], qt] = [
                            ps_ctx.tile([P, 512], F32, tag="ctx", name=f"cps{i}")
                            for i in range(len(hs))
                        ]
                    c_ps = c_ps_of[hs[0], qt]
                    final = k1 == nkc
                    if final:
                        rdens = [
                            rdenX[2 * (unit_ctr[0] % 2) + i]
                            for i in range(len(hs))
                        ]
                        unit_ctr[0] += 1
                    # head-serial: head i's ctx and reciprocal go out before
                    # head i+1's ctx, so the 3.3us DVE reciprocal overlaps
                    # the other head's ctx matmuls; the rest of the normalize
                    # (emit_norm) is deferred one push unit so the broadcast
                    # matmul never stalls the in-order PE queue
                    for i, h in enumerate(hs):
                        for kc in range(k0, k1):
                            j = kc - 4 * qt
                            pT = p_tiles[kc - kc % 2]
                            coff = max(0, j) * P
                            nc.tensor.matmul(
                                c_ps[i][0 : HD + 1, coff:512],
                                v_sb[:, kc, h, 0 : HD + 1],
                                pT[:, kc % 2, i, coff:512],
                                start=(kc == 0),
                                stop=(kc == nkc - 1),
                            )
                    if final:
                        norm_pend.append((hs, qt, rdens, tail))

                def emit_norm(hs, qt, rdens, tail=False):
                    # normalize: divide by the ones-row sums, broadcast
                    # across partitions on gpsimd (keeps the PE queue out of
                    # the reciprocal's shadow entirely); then the A2A sends
                    # (destination block qt, duplicated across batch halves)
                    # and on qt3 head h's collective
                    c_ps = c_ps_of.pop((hs[0], qt))
                    # reciprocals first (both heads), so the second head's
                    # runs under the first head's broadcast+normalize; emitted
                    # here (one push after ctx) so they sit BEHIND the next
                    # chunk's diag masks in the DVE FIFO instead of blocking
                    # them
                    for i in range(len(hs)):
                        rX = rdens[i]
                        if tail:
                            # final flush gates the last collective triggers
                            # and ACT is idle there: reciprocal as exp(-ln)
                            # on ACT, off the DVE queue
                            lnd = smallp.tile([P, 512], F32, tag="bb")
                            nc.scalar.activation(
                                lnd[64:65, :],
                                c_ps[i][64:65, :],
                                mybir.ActivationFunctionType.Ln,
                            )
                            nc.scalar.activation(
                                rX[64:65, :], lnd[64:65, :], Exp, scale=-1.0
                            )
                        else:
                            with nc.allow_low_precision(reason="den recip"):
                                nc.vector.reciprocal(
                                    rX[64:65, :], c_ps[i][64:65, :]
                                )
                    for i, h in enumerate(hs):
                        # f32r broadcast matmul: 1 cycle/row vs fp32's 4; the
                        # normalize TT stages via a DVE copy (one PSUM
                        # operand per TT)
                        b_ps = ps_mm.tile([P, 2, 512], F32, tag="mm", name="bps")[
                            :, 0, :
                        ]
                        nc.tensor.matmul(
                            b_ps[0:HD, :],
                            onescol_r[:, 0:HD],
                            rdens[i][:],
                            start=True,
                            stop=True,
                        )
                        bb = smallp.tile([HD, 512], F32, tag="bb")
                        nc.vector.tensor_copy(bb[:], b_ps[0:HD, :])
                        nc.vector.tensor_tensor(
                            ctxn[h][qt][:, :],
                            c_ps[i][0:HD, :],
                            bb[:],
                            MULT,
                        )
                        for sh in (qt, qt + 4):
                            if h < 2:
                                nc.sync.dma_start(
                                    a2a_in[h][sh * HD : (sh + 1) * HD, :],
                                    ctxn[h][qt][:, :],
                                )
                            else:
                                nc.sync.dma_start(
                                    a2a_in[2][
                                        sh * P
                                        + (h % 2) * HD : sh * P
                                        + (h % 2 + 1) * HD,
                                        :,
                                    ],
                                    ctxn[h][qt][:, :],
                                )
                        if qt == NQT - 1 and coll and h != 2:
                            bi = min(h, 2)
                            nc.gpsimd.collective_compute(
                                "AllToAll",
                                mybir.AluOpType.bypass,
                                ins=[a2a_in[bi].opt()],
                                outs=[a2a_out[bi].opt()],
                                replica_groups=[list(range(NC))],
                            )

                def qt_chunks(qt):
                    return [
                        (qt, kcb, min(kcb + 8, 4 * qt + 4))
                        for kcb in range(0, 4 * qt + 4, 8)
                    ]

                # software pipeline: emit scores(i+1) before ctx(i) so the PE
                # queue never blocks the ACT exp stream behind ctx matmuls;
                # norms run one more push behind so reciprocals are long done;
                # pair-0 attention interleaves with the QKV q-block loop
                pend = []
                norm_pend = []

                def push(pair, ch):
                    tiles = emit_scores(pair, ch)
                    # only norms queued BEFORE this push (one-unit deferral)
                    ready_norms = norm_pend[:]
                    del norm_pend[:]
                    if pend:
                        emit_ctx(*pend.pop())
                    for n in ready_norms:
                        emit_norm(*n)
                    pend.append((pair, ch, tiles))

                # pair-0 (heads 0,1 together) rides the QKV block loop; its
                # qt3 -- and A2A0/A2A1 -- completes right after QKV(3), so
                # those collectives hide under heads 2-3's attention. Heads
                # 2 and 3 then run as single-head passes: A2A2 fires a full
                # head-pass (~25us) before the end, leaving only A2A3
                # (0.5MB) exposed at the tail.
                for g in range(NQT):
                    emit_qkv(g)
                    for ch in qt_chunks(g):
                        push((0, 1), ch)
                emit_ctx(*pend.pop())
                for qt in range(NQT):
                    for ch in qt_chunks(qt):
                        push((2, 3), ch)
                _fp, _fc, _ft = pend.pop()
                emit_ctx(_fp, _fc, _ft, tail=True)
                for n in norm_pend:
                    emit_norm(*n)
                del norm_pend[:]

                smallp.release()
                pTp.release()
                work.release()

            # ---------- receive + output projection ----------
            # per-head receive for heads 0/1: a2a_out[h] rows = 8 senders x
            # 64 dims; jj blocks of 128 rows pair senders (2jj, 2jj+1), jj
            # 0-1 batch 0, jj 2-3 batch 1
            outp = tc.alloc_tile_pool(name="outp", bufs=1)
            gsrc = a2a_out if coll else a2a_in
            cx = outp.tile([P, 2, 4, TB], BF16, tag="cx")
            tmp = outp.tile([P, 2, TB], BF16, tag="seltmp")
            # ctxf[p, h, jj, t]: wo_sb chunk h*2+jj (Wo rows host-permuted)
            ctxf = outp.tile([P, 2, 2, TB], BF16, tag="ctxf")

            def recv_head(h):
                for hh in range(2):
                    hc = slice(hh * 256, (hh + 1) * 256)
                    nc.sync.dma_start(
                        cx[:, h, :, hc],
                        gsrc[h][:, hc].rearrange("(j q) t -> q j t", q=P),
                    )
                    nc.vector.tensor_scalar(
                        tmp[:, :, hc], cx[:, h, 0:2, hc], bsel_sb[:, 0:1],
                        None, MULT,
                    )
                    nc.vector.scalar_tensor_tensor(
                        ctxf[:, h, :, hc], cx[:, h, 2:4, hc],
                        bsel_sb[:, 1:2], tmp[:, :, hc], MULT, ADD,
                    )

            with tc.tile_pool(name="out_pool", bufs=3) as out_pool:
                o_parts = [
                    outp.tile([P, 512], F32, tag=f"opart{u}", name=f"opart{u}")
                    for u in range(8)
                ]
                # heads 0-1 partials: overlap with the pair-1 A2A
                recv_head(0)
                recv_head(1)
                for u in range(8):
                    tc_i, nt = u // 2, u % 2
                    po = ps_mm.tile([P, 2, 512], F32, tag="mm")
                    for i, (h, jj) in enumerate(
                        [(0, 0), (0, 1), (1, 0), (1, 1)]
                    ):
                        nc.tensor.matmul(
                            po[:, 0, :],
                            ctxf[:, h, jj, tc_i * P : (tc_i + 1) * P],
                            wo_sb[:, h * 2 + jj, nt * 512 : (nt + 1) * 512],
                            start=(i == 0),
                            stop=(i == 3),
                        )
                    nc.vector.tensor_tensor(
                        o_parts[u][:],
                        po[:, 0, :],
                        bo_bc[:, nt * 512 : (nt + 1) * 512],
                        ADD,
                    )
                # keep-warm: dummy matmuls bridging to the pair-1 A2A so the
                # PE doesn't re-throttle (>3us idle drops HAM to half clock)
                # before the pair-1 output projection
                warm = ps_mm.tile([P, 2, 512], F32, tag="mm", name="warm")
                for wj in range(32):
                    nc.tensor.matmul(
                        warm[:, 0, :],
                        ctxf[:, 0, wj % 2, 0:P],
                        wo_sb[:, 0, 0:512],
                        start=True,
                        stop=True,
                    )
                # pair-1 (heads 2+3) merged receive: rows = 8 senders x 128
                # dims; sender blocks j 0-3 batch 0, 4-7 batch 1. Receive,
                # select and project per 128-col token quarter so the first
                # po chains start ~2.5us after the collective lands instead
                # of waiting for the full 1MB readback
                cx1 = outp.tile([P, NC, TB], BF16, tag="cx1")
                tmp1 = outp.tile([P, NQT, TB], BF16, tag="seltmp1")
                ctxf1 = outp.tile([P, NQT, TB], BF16, tag="ctxf1")
                g1 = gsrc[2]
                for tc_i in range(4):
                    hc = slice(tc_i * P, (tc_i + 1) * P)
                    nc.sync.dma_start(
                        cx1[:, :, hc],
                        g1[:, hc].rearrange("(j q) t -> q j t", q=P),
                    )
                    nc.vector.tensor_scalar(
                        tmp1[:, :, hc], cx1[:, 0:4, hc], bsel_sb[:, 0:1],
                        None, MULT,
                    )
                    nc.vector.scalar_tensor_tensor(
                        ctxf1[:, :, hc], cx1[:, 4:8, hc],
                        bsel_sb[:, 1:2], tmp1[:, :, hc], MULT, ADD,
                    )
                    for nt in range(2):
                        u = tc_i * 2 + nt
                        po = ps_mm.tile([P, 2, 512], F32, tag="mm")
                        for g in range(NQT):
                            nc.tensor.matmul(
                                po[:, 0, :],
                                ctxf1[:, g, hc],
                                wo_sb[:, 4 + g, nt * 512 : (nt + 1) * 512],
                                start=(g == 0),
                                stop=(g == NQT - 1),
                            )
                        o_sb = out_pool.tile([P, 512], F32, tag="osb")
                        nc.vector.tensor_tensor(
                            o_sb[:], po[:, 0, :], o_parts[u][:], ADD
                        )
                        nc.sync.dma_start(
                            y_d[
                                tc_i * P : (tc_i + 1) * P,
                                nt * 512 : (nt + 1) * 512,
                            ],
                            o_sb[:],
                        )

            outp.release()

    nc.compile()
    return nc


_NC_CACHE = {}


def _get_nc():
    if "nc" not in _NC_CACHE:
        _NC_CACHE["nc"] = build()
    return _NC_CACHE["nc"]


def _make_in_maps(x, Wq, bq, Wk, bk, Wv, bv, Wo, bo):
    x = np.asarray(x, np.float32)
    Wq, Wk, Wv, Wo = (np.asarray(a, np.float32) for a in (Wq, Wk, Wv, Wo))
    bq, bk, bv, bo = (np.asarray(a, np.float32) for a in (bq, bk, bv, bo))
    np_dx = NP_F8 if FP8_QKV else NP_BF16
    # permute Wo rows to the receive order: chunks 0-3 = per-head (h, jj)
    # layout for heads 0/1; chunks 4-7 = merged pair-1 (sender g) layout
    perm = np.empty(D, np.int64)
    for ci in range(4):
        h, jj = ci // 2, ci % 2
        for par in range(2):
            H = (2 * jj + par) * 4 + h
            base = ci * P + par * HD
            perm[base : base + HD] = np.arange(H * HD, (H + 1) * HD)
    for g in range(4):
        for par in range(2):
            H = 4 * g + 2 + par
            base = (4 + g) * P + par * HD
            perm[base : base + HD] = np.arange(H * HD, (H + 1) * HD)
    wo_b = np.ascontiguousarray(Wo[perm]).astype(NP_BF16)
    in_maps = []
    for c in range(NC):
        b, g = c // 4, c % 4
        sl = slice(g * G, (g + 1) * G)
        bsel = np.tile(
            np.array([1.0 - b, float(b)], np.float32).reshape(1, 2), (P, 1)
        )
        in_maps.append(
            {
                "xt": np.ascontiguousarray(x[b].T).astype(np_dx),
                "wq": np.ascontiguousarray(Wq[:, sl] * WSC).astype(np_dx),
                "wk": np.ascontiguousarray(Wk[:, sl] * WSC).astype(np_dx),
                "wv": np.ascontiguousarray(Wv[:, sl] * WSC).astype(np_dx),
                "xt0": np.ascontiguousarray(x[b].T[:, :TB]).astype(NP_BF16),
                "wq0": np.ascontiguousarray(Wq[:, sl]).astype(NP_BF16),
                "wk0": np.ascontiguousarray(Wk[:, sl]).astype(NP_BF16),
                "wv0": np.ascontiguousarray(Wv[:, sl]).astype(NP_BF16),
                "bq": np.ascontiguousarray(bq[sl].reshape(MC, P).T),
                "bk": np.ascontiguousarray(bk[sl].reshape(MC, P).T),
                "bv": np.ascontiguousarray(bv[sl].reshape(1, G)),
                "wo": wo_b,
                "bo": np.ascontiguousarray(bo.reshape(1, D)),
                "bsel": np.ascontiguousarray(bsel),
            }
        )
    return in_maps


def run(inputs, trace=False, tmpdir=None):
    """Run on 8 cores; returns (output [2,2048,1024], BassKernelResults)."""
    if trace:
        _install_ntff_hook()
    nc = _get_nc()
    in_maps = _make_in_maps(**inputs)
    res = bass_utils.run_bass_kernel_spmd(
        nc, in_maps, core_ids=list(range(NC)), trace=trace, tmpdir=tmpdir
    )
    out = np.empty((B, S, D), np.float32)
    for c in range(NC):
        b, g = c // 4, c % 4
        out[b, g * TB : (g + 1) * TB, :] = res.results[c]["y"]
    return out, res


def kernel(**inputs) -> np.ndarray:
    out, _ = run(inputs, trace=False)
    return out



# revision 47
# speedup vs baseline: 1.2899x; 1.0610x over previous
"""GPT self-attention layer (B=2, S=2048, D=1024, H=16, hd=64) on 8 TRN2 cores.

Sharding: data-parallel over batch (2) x tensor-parallel over heads (4 groups
of 4 heads). Core c handles batch b=c//4, head group g=c%4.

Per-core pipeline (bf16 attention, fp8-DoubleRow projections):
  1. QKV projections per 512-token block: fp8e4m3 x/W inputs with DoubleRow
     matmuls (4x PE rate) for blocks 1-3; block 0 takes a bf16 side path
     (xt0/wq0/wk0/wv0 inputs) because the ~3.5% fp8 x*W value error is
     amplified by the short causal softmax rows that all live there. fp8
     weights are host-prescaled by 64 (subnormal dodge) and descaled for
     free in the ACT bias op / DVE V-store. q/k/v stored bf16; V carries an
     interleaved ones column so the softmax denominator accumulates in ctx
     psum row 64 for free.
  2. Attention per head pair, scoresT [k-part, q-free], exp on ACT into
     bf16 pT (kc-pair tiles), causal diag masked by a DVE multiply, ctx
     accumulated per kc head-serial. 1/den via DVE reciprocal (row 64),
     broadcast across partitions by an f32r matmul against a
     ones-at-row-64 column (1 cycle/row vs fp32's 4), DVE-staged
     normalize -> bf16.
  3. Emission is software-pipelined three ways: scores(chunk i+1) go out
     before ctx(chunk i) so ctx never blocks the ACT exp stream; each
     unit's normalize (reciprocal included) is deferred one further push
     so the 3.3us DVE reciprocal sits behind the next chunk's diag masks
     in the DVE FIFO and its broadcast matmul never stalls the in-order
     PE queue; pair-0 rides the QKV block loop so its qt3 completes right
     after QKV(3).
  4. Three bf16 AllToAlls, shards duplicated across batch halves: two
     per-head 0.5MB ones for heads 0/1, fired right after QKV(3) and
     hidden under all of pair-1's attention, and one merged 1MB one for
     heads 2+3 at the end (a single transfer beats two serialized ones
     there). gpsimd holds only collectives, so the waiting triggers stall
     nothing. Receiver picks its batch half with a DVE mask-select driven
     by the bsel input (no gather).
  5. out = ctxT.T @ Wo + bo in bf16, Wo host-permuted to the receive
     order (per-head chunks 0-3, paired chunks 4-7): heads 0/1 partial
     sums plus keep-warm matmuls (HAM re-throttles after >3us PE idle)
     run under the pair-1 AllToAll; its receive+select+projection is
     pipelined per 128-col token quarter; DMA to y [512,1024] fp32.
     Startup block-0 loads are split per dc chunk so the first q matmul
     starts as soon as its first slices land.
"""

import contextlib
import ctypes
import sys
import types

sys.path.insert(0, "/opt/trn_rl_repo")

import numpy as np
import ml_dtypes

import concourse.bass as bass
import concourse.mybir as mybir
import concourse.tile as tile
from concourse import bacc
from concourse import bass_utils

P = 128
B, S, D = 2, 2048, 1024
NH_LOC = 4          # heads per core
HD = 64             # head dim
G = NH_LOC * HD     # local head dims = 256
MC = G // P         # m-chunks of local dims = 2
DC = D // P         # d-chunks = 8
TB = 512            # token block (output tokens per core, q-tile width)
NQT = S // TB       # q-tiles = 4
NTC = S // P        # token chunks = 16
NC = 8
VP = 80             # per-head v block stride (64 data + 1 ones + 15 pad)

F32 = mybir.dt.float32
F32R = mybir.dt.float32r
BF16 = mybir.dt.bfloat16
F8 = mybir.dt.float8e4
Exp = mybir.ActivationFunctionType.Exp
Ident = mybir.ActivationFunctionType.Identity
MULT = mybir.AluOpType.mult
ADD = mybir.AluOpType.add
DR = mybir.MatmulPerfMode.DoubleRow

# per-stage dtype switches (bf16 fallback for numerics bisection).
# FP8_QKV runs the q/k/v projections in fp8 DoubleRow (4x PE) for token
# blocks 1-3; block 0 always takes a bf16 side path because the fp8 x*W
# value error (~3.5%) is amplified by the short causal softmax rows that
# all live in the first block.
FP8_QKV = True
FP8_SC = True
FP8_CTX = False
WSC = 64.0 if FP8_QKV else 1.0  # host weight prescale

DX = F8 if FP8_QKV else BF16    # xt, wq/wk/wv
DQK = F8 if FP8_SC else BF16    # stored q / k
DP = F8 if FP8_CTX else BF16    # pT probs and v

NP_BF16 = np.dtype(ml_dtypes.bfloat16)
NP_F8 = np.dtype(ml_dtypes.float8_e4m3)


def _install_ntff_hook():
    """Make trace=True work under axon: inject antenv.axon_hooks backed by
    ctypes calls into libaxon_pjrt.so (mirrors trn_agent_boot logic)."""
    if "antenv.axon_hooks" in sys.modules:
        return
    holder = {}
    mod = types.ModuleType("antenv.axon_hooks")
    mod.set_axon_ntff_profile_hook = lambda h: holder.update(h=h)
    mod.get_axon_ntff_profile_hook = lambda: holder.get("h")
    sys.modules["antenv.axon_hooks"] = mod
    try:
        lib = ctypes.CDLL("/opt/axon/libaxon_pjrt.so")
        if not hasattr(lib, "axon_start_nrt_profile"):
            return
    except OSError:
        return
    lib.axon_start_nrt_profile.argtypes = [
        ctypes.POINTER(ctypes.c_int64),
        ctypes.c_size_t,
    ]
    lib.axon_start_nrt_profile.restype = ctypes.c_int64
    lib.axon_stop_nrt_profile.argtypes = [ctypes.c_char_p]
    lib.axon_stop_nrt_profile.restype = ctypes.c_int64

    @contextlib.contextmanager
    def _hook(output_dir, device_ids):
        import jax

        jax.devices()
        if device_ids:
            ids = (ctypes.c_int64 * len(device_ids))(*device_ids)
            rc = lib.axon_start_nrt_profile(ids, len(device_ids))
        else:
            rc = lib.axon_start_nrt_profile(None, 0)
        if rc != 0:
            raise RuntimeError(f"axon_start_nrt_profile rc={rc}")
        try:
            yield
        finally:
            n = lib.axon_stop_nrt_profile(str(output_dir).encode())
            print(f"profile: {n} ntff file(s) written to {output_dir}")

    holder["h"] = _hook


def build(coll=True):
    nc = bacc.Bacc("TRN2", target_bir_lowering=False, debug=False, num_devices=NC)

    xt_d = nc.dram_tensor("xt", [D, S], DX, kind="ExternalInput").ap()
    wq_d = nc.dram_tensor("wq", [D, G], DX, kind="ExternalInput").ap()
    wk_d = nc.dram_tensor("wk", [D, G], DX, kind="ExternalInput").ap()
    wv_d = nc.dram_tensor("wv", [D, G], DX, kind="ExternalInput").ap()
    xt0_d = nc.dram_tensor("xt0", [D, TB], BF16, kind="ExternalInput").ap()
    wq0_d = nc.dram_tensor("wq0", [D, G], BF16, kind="ExternalInput").ap()
    wk0_d = nc.dram_tensor("wk0", [D, G], BF16, kind="ExternalInput").ap()
    wv0_d = nc.dram_tensor("wv0", [D, G], BF16, kind="ExternalInput").ap()
    bq_d = nc.dram_tensor("bq", [P, MC], F32, kind="ExternalInput").ap()
    bk_d = nc.dram_tensor("bk", [P, MC], F32, kind="ExternalInput").ap()
    bv_d = nc.dram_tensor("bv", [1, G], F32, kind="ExternalInput").ap()
    wo_d = nc.dram_tensor("wo", [D, D], BF16, kind="ExternalInput").ap()
    bo_d = nc.dram_tensor("bo", [1, D], F32, kind="ExternalInput").ap()
    bsel_d = nc.dram_tensor("bsel", [P, 2], F32, kind="ExternalInput").ap()
    y_d = nc.dram_tensor("y", [TB, D], F32, kind="ExternalOutput").ap()

    with tile.TileContext(nc) as tc:
        with (
            tc.tile_pool(name="const", bufs=1) as const,
            tc.tile_pool(name="dram", bufs=1, space="DRAM") as dram,
            tc.tile_pool(name="ps_mm", bufs=2, space="PSUM") as ps_mm,
            tc.tile_pool(name="ps_ctx", bufs=4, space="PSUM") as ps_ctx,
            tc.tile_pool(name="persist", bufs=1) as persist,
        ):
            # ---------------- constants ----------------
            ones_f = const.tile([P, 1], F32, tag="ones_f")
            nc.vector.memset(ones_f[:], 1.0)
            # trimask[k, u] = 1 if k <= u else 0 (keep where u - k >= 0)
            tri_f = const.tile([P, P], F32, tag="tri_f")
            nc.gpsimd.memset(tri_f[:], 1.0)
            nc.gpsimd.affine_select(
                out=tri_f[:],
                in_=tri_f[:],
                compare_op=mybir.AluOpType.is_ge,
                fill=0.0,
                base=0,
                pattern=[[1, P]],
                channel_multiplier=-1,
            )
            # materialized for both heads so the gpsimd mask multiply uses a
            # plain strided AP (no broadcast)
            tri_p = const.tile([P, 2, P], DP, tag="tri_p")
            nc.vector.tensor_copy(
                tri_p[:], tri_f[:, None, :].to_broadcast((P, 2, P))
            )
            # ones at row 64 only: broadcast-den matmul weights
            zrow_f = const.tile([P, HD], F32, tag="zrow_f")
            nc.vector.memset(zrow_f[:], 0.0)
            nc.vector.memset(zrow_f[64:65, :], 1.0)
            onescol_r = const.tile([P, HD], F32R, tag="onescol_r")
            nc.vector.tensor_copy(onescol_r[:], zrow_f[:])
            zeros_f = const.tile([P, 512], F32, tag="zeros_f")
            nc.vector.memset(zeros_f[:], 0.0)

            bq_sb = const.tile([P, MC], F32, tag="bq")
            bk_sb = const.tile([P, MC], F32, tag="bk")
            nc.sync.dma_start(bq_sb[:], bq_d)
            nc.sync.dma_start(bk_sb[:], bk_d)
            bv_row = const.tile([1, G], F32, tag="bv_row")
            nc.sync.dma_start(bv_row[:], bv_d)
            bv_bc = const.tile([P, G], F32, tag="bv_bc")
            nc.gpsimd.partition_broadcast(bv_bc[:], bv_row[:])
            bo_row = const.tile([1, D], F32, tag="bo_row")
            bo_bc = const.tile([P, D], F32, tag="bo_bc")
            bsel_sb = const.tile([P, 2], F32, tag="bsel")
            nc.sync.dma_start(bsel_sb[:], bsel_d)

            # persistent activations: q/k [p, pair, kslab, tok] with kslab 1
            # zeroed (DoubleRow zero-pad), v [p, tc, head, 80] fp8
            KSL = 2 if FP8_SC else 1
            qT = persist.tile([P, MC, KSL, S], DQK, tag="qT")
            kT = persist.tile([P, MC, KSL, S], DQK, tag="kT")
            v_sb = persist.tile([P, NTC, NH_LOC, VP], DP, tag="v")
            wo_sb = persist.tile([P, DC, D], BF16, tag="wo")

            if FP8_SC:
                nc.gpsimd.memset(qT[:, :, 1, :], 0.0)
                nc.gpsimd.memset(kT[:, :, 1, :], 0.0)
            # ones column of v (denominator trick): col 64 of each head block
            nc.gpsimd.memset(v_sb[:, :, :, HD : HD + 1], 1.0)

            # heads 0/1 get per-head 0.5MB buffers (their collectives hide
            # under pair-1 attention); heads 2/3 share one 1MB buffer so the
            # end-gated exchange is a single transfer, not two serialized
            a2a_in = [
                dram.tile([NC * HD, TB], BF16, name="a2ain0", tag="a2ain0"),
                dram.tile([NC * HD, TB], BF16, name="a2ain1", tag="a2ain1"),
                dram.tile([NC * P, TB], BF16, name="a2ainp1", tag="a2ainp1"),
            ]
            a2a_out = [
                dram.tile([NC * HD, TB], BF16, name="a2aout0", tag="a2aout0"),
                dram.tile([NC * HD, TB], BF16, name="a2aout1", tag="a2aout1"),
                dram.tile([NC * P, TB], BF16, name="a2aoutp1", tag="a2aoutp1"),
            ]

            with (
                tc.tile_pool(name="xw", bufs=1) as xw,
            ):
                wq_sb = xw.tile([P, DC, G], DX, tag="wq")
                wk_sb = xw.tile([P, DC, G], DX, tag="wk")
                wv_sb = xw.tile([P, DC, G], DX, tag="wv")
                # bf16 block-0 path: loaded first since QKV(0) starts the
                # whole pipeline
                wq0_sb = xw.tile([P, DC, G], BF16, tag="wq0")
                wk0_sb = xw.tile([P, DC, G], BF16, tag="wk0")
                wv0_sb = xw.tile([P, DC, G], BF16, tag="wv0")
                x0_sb = xw.tile([P, DC, TB], BF16, tag="x0")
                # per-dc-chunk DMAs: the first q matmul chain only waits for
                # the dc slices it has consumed so far, not the whole tile
                wq0_r = wq0_d.rearrange("(dc p) m -> p dc m", p=P)
                x0_r = xt0_d.rearrange("(dc p) t -> p dc t", p=P)
                for dc in range(DC):
                    nc.sync.dma_start(wq0_sb[:, dc, :], wq0_r[:, dc, :])
                    nc.sync.dma_start(x0_sb[:, dc, :], x0_r[:, dc, :])
                nc.sync.dma_start(
                    wk0_sb[:], wk0_d.rearrange("(dc p) m -> p dc m", p=P)
                )
                nc.sync.dma_start(
                    wv0_sb[:], wv0_d.rearrange("(dc p) m -> p dc m", p=P)
                )
                nc.sync.dma_start(wq_sb[:], wq_d.rearrange("(dc p) m -> p dc m", p=P))

                xTt = [
                    xw.tile([P, DC, TB], DX, tag=f"xT{g}", name=f"xT{g}")
                    for g in range(1, NQT)
                ]
                xTg = [None] + [t[:] for t in xTt]
                xt_r = xt_d.rearrange("(dc p) t -> p dc t", p=P)

                for g in range(1, NQT):
                    nc.sync.dma_start(
                        xTg[g], xt_r[:, :, g * TB : (g + 1) * TB]
                    )
                    if g == 1:
                        nc.sync.dma_start(
                            wk_sb[:], wk_d.rearrange("(dc p) m -> p dc m", p=P)
                        )
                        nc.sync.dma_start(
                            wv_sb[:], wv_d.rearrange("(dc p) m -> p dc m", p=P)
                        )

                def emit_qkv(g):
                    if FP8_QKV and g > 0:
                        wq_g, wk_g, wv_g, xg = wq_sb, wk_sb, wv_sb, xTg[g]
                        sc, use_dr = 1.0 / WSC, True
                    else:
                        wq_g, wk_g, wv_g, xg = wq0_sb, wk0_sb, wv0_sb, x0_sb[:]
                        sc, use_dr = 1.0, False
                    for w_sb, b_sb, out_t in ((wq_g, bq_sb, qT), (wk_g, bk_sb, kT)):
                        for mc_i in range(MC):
                            pj = ps_mm.tile(
                                [P, 2, 512], F32, tag="mm", name="pj"
                            )[:, 0, :]
                            if use_dr:
                                for i, dc in enumerate(range(0, DC, 2)):
                                    nc.tensor.matmul(
                                        pj[:],
                                        w_sb[:, dc : dc + 2, mc_i * P : (mc_i + 1) * P],
                                        xg[:, dc : dc + 2, :],
                                        start=(i == 0),
                                        stop=(dc == DC - 2),
                                        perf_mode=DR,
                                    )
                            else:
                                for dc in range(DC):
                                    nc.tensor.matmul(
                                        pj[:],
                                        w_sb[:, dc, mc_i * P : (mc_i + 1) * P],
                                        xg[:, dc, :],
                                        start=(dc == 0),
                                        stop=(dc == DC - 1),
                                    )
                            nc.scalar.activation(
                                out_t[:, mc_i, 0, g * TB : (g + 1) * TB],
                                pj[:],
                                Ident,
                                bias=b_sb[:, mc_i : mc_i + 1],
                                scale=sc,
                            )
                    for ti in range(4):
                        tc_i = 4 * g + ti
                        pv = ps_mm.tile(
                            [P, 2, 512], F32, tag="mm", name="pv"
                        )[:, 0, :]
                        if use_dr:
                            for i, dc in enumerate(range(0, DC, 2)):
                                nc.tensor.matmul(
                                    pv[:, 0:G],
                                    xg[:, dc : dc + 2, ti * P : (ti + 1) * P],
                                    wv_g[:, dc : dc + 2, :],
                                    start=(i == 0),
                                    stop=(dc == DC - 2),
                                    perf_mode=DR,
                                )
                        else:
                            for dc in range(DC):
                                nc.tensor.matmul(
                                    pv[:, 0:G],
                                    xg[:, dc, ti * P : (ti + 1) * P],
                                    wv_g[:, dc, :],
                                    start=(dc == 0),
                                    stop=(dc == DC - 1),
                                )
                        # v = pv*sc + bv into padded head blocks
                        nc.vector.scalar_tensor_tensor(
                            v_sb[:, tc_i, :, 0:HD],
                            pv[:, 0:G].rearrange("p (h c) -> p h c", c=HD),
                            sc,
                            bv_bc[:].rearrange("p (h c) -> p h c", c=HD),
                            MULT,
                            ADD,
                        )

                # ------ attention, emission-interleaved with QKV blocks ------
                nc.sync.dma_start(
                    wo_sb[:], wo_d.rearrange("(dc p) n -> p dc n", p=P)
                )
                nc.sync.dma_start(bo_row[:], bo_d)
                nc.gpsimd.partition_broadcast(bo_bc[:], bo_row[:])
                work = tc.alloc_tile_pool(name="att", bufs=1)
                pTp = tc.alloc_tile_pool(name="pTp", bufs=8)
                smallp = tc.alloc_tile_pool(name="smallp", bufs=3)
                ctxn = [
                    [
                        work.tile(
                            [HD, TB], BF16, tag=f"ctxn{h}_{q}", name=f"ctxn{h}_{q}"
                        )
                        for q in range(NQT)
                    ]
                    for h in range(NH_LOC)
                ]
                # reciprocal of denominator lives at row 64; other rows stay 0
                # so the broadcast matmul (ones at row 64) reads no garbage.
                # double-buffered by qt-unit parity: the deferred norm of unit
                # i reads its rden while unit i+1's reciprocal writes the other
                rdenX = [
                    work.tile([P, 512], F32R, tag=f"rdenX{i}", name=f"rdenX{i}")
                    for i in range(4)
                ]
                for i in range(4):
                    nc.vector.tensor_copy(rdenX[i][:], zeros_f[:])
                c_ps_of = {}
                unit_ctr = [0]

                def emit_scores(hs, ch):
                    qt, k0, k1 = ch
                    nh = len(hs)
                    p_tiles = {}
                    for kc in range(k0, k1):
                        j = kc - 4 * qt
                        coff = max(0, j) * P
                        if kc % 2 == 0:
                            pT = pTp.tile([P, 2, nh, TB], DP, tag=f"pT{nh}")
                            p_tiles[kc] = pT
                        else:
                            pT = p_tiles[kc - 1]
                        s_ps = ps_mm.tile([P, 2, 512], F32, tag="mm")
                        for i, h in enumerate(hs):
                            pb = (h % 2) * HD
                            nc.tensor.matmul(
                                s_ps[:, i, coff:512],
                                kT[
                                    pb : pb + HD, h // 2, 0, kc * P : (kc + 1) * P
                                ],
                                qT[
                                    pb : pb + HD,
                                    h // 2,
                                    0,
                                    qt * TB + coff : (qt + 1) * TB,
                                ],
                                start=True,
                                stop=True,
                            )
                        nc.scalar.activation(
                            pT[:, kc % 2, :, coff:512],
                            s_ps[:, 0:nh, coff:512],
                            Exp,
                            scale=0.125,
                        )
                        if j >= 0:
                            nc.vector.tensor_tensor(
                                pT[:, kc % 2, :, coff : coff + P],
                                pT[:, kc % 2, :, coff : coff + P],
                                tri_p[:, 0:nh, :],
                                MULT,
                            )
                    return p_tiles

                def emit_ctx(hs, ch, p_tiles, tail=False):
                    qt, k0, k1 = ch
                    nkc = 4 * qt + 4
                    if k0 == 0:
                        c_ps_of[hs[0], qt] = [
                            ps_ctx.tile([P, 512], F32, tag="ctx", name=f"cps{i}")
                            for i in range(len(hs))
                        ]
                    c_ps = c_ps_of[hs[0], qt]
                    final = k1 == nkc
                    if final:
                        rdens = [
                            rdenX[2 * (unit_ctr[0] % 2) + i]
                            for i in range(len(hs))
                        ]
                        unit_ctr[0] += 1
                    # head-serial: head i's ctx and reciprocal go out before
                    # head i+1's ctx, so the 3.3us DVE reciprocal overlaps
                    # the other head's ctx matmuls; the rest of the normalize
                    # (emit_norm) is deferred one push unit so the broadcast
                    # matmul never stalls the in-order PE queue
                    for i, h in enumerate(hs):
                        for kc in range(k0, k1):
                            j = kc - 4 * qt
                            pT = p_tiles[kc - kc % 2]
                            coff = max(0, j) * P
                            nc.tensor.matmul(
                                c_ps[i][0 : HD + 1, coff:512],
                                v_sb[:, kc, h, 0 : HD + 1],
                                pT[:, kc % 2, i, coff:512],
                                start=(kc == 0),
                                stop=(kc == nkc - 1),
                            )
                    if final:
                        norm_pend.append((hs, qt, rdens, tail))

                def emit_norm(hs, qt, rdens, tail=False):
                    # normalize: divide by the ones-row sums, broadcast
                    # across partitions on gpsimd (keeps the PE queue out of
                    # the reciprocal's shadow entirely); then the A2A sends
                    # (destination block qt, duplicated across batch halves)
                    # and on qt3 head h's collective
                    c_ps = c_ps_of.pop((hs[0], qt))
                    # reciprocals first (both heads), so the second head's
                    # runs under the first head's broadcast+normalize; emitted
                    # here (one push after ctx) so they sit BEHIND the next
                    # chunk's diag masks in the DVE FIFO instead of blocking
                    # them
                    for i in range(len(hs)):
                        rX = rdens[i]
                        if tail:
                            # final flush gates the last collective triggers
                            # and ACT is idle there: reciprocal as exp(-ln)
                            # on ACT, off the DVE queue
                            lnd = smallp.tile([P, 512], F32, tag="bb")
                            nc.scalar.activation(
                                lnd[64:65, :],
                                c_ps[i][64:65, :],
                                mybir.ActivationFunctionType.Ln,
                            )
                            nc.scalar.activation(
                                rX[64:65, :], lnd[64:65, :], Exp, scale=-1.0
                            )
                        else:
                            with nc.allow_low_precision(reason="den recip"):
                                nc.vector.reciprocal(
                                    rX[64:65, :], c_ps[i][64:65, :]
                                )
                    for i, h in enumerate(hs):
                        # f32r broadcast matmul: 1 cycle/row vs fp32's 4; the
                        # normalize TT stages via a DVE copy (one PSUM
                        # operand per TT)
                        b_ps = ps_mm.tile([P, 2, 512], F32, tag="mm", name="bps")[
                            :, 0, :
                        ]
                        nc.tensor.matmul(
                            b_ps[0:HD, :],
                            onescol_r[:, 0:HD],
                            rdens[i][:],
                            start=True,
                            stop=True,
                        )
                        bb = smallp.tile([HD, 512], F32, tag="bb")
                        nc.vector.tensor_copy(bb[:], b_ps[0:HD, :])
                        nc.vector.tensor_tensor(
                            ctxn[h][qt][:, :],
                            c_ps[i][0:HD, :],
                            bb[:],
                            MULT,
                        )
                        for sh in (qt, qt + 4):
                            if h < 2:
                                nc.sync.dma_start(
                                    a2a_in[h][sh * HD : (sh + 1) * HD, :],
                                    ctxn[h][qt][:, :],
                                )
                            else:
                                nc.sync.dma_start(
                                    a2a_in[2][
                                        sh * P
                                        + (h % 2) * HD : sh * P
                                        + (h % 2 + 1) * HD,
                                        :,
                                    ],
                                    ctxn[h][qt][:, :],
                                )
                        if qt == NQT - 1 and coll and h != 2:
                            bi = min(h, 2)
                            nc.gpsimd.collective_compute(
                                "AllToAll",
                                mybir.AluOpType.bypass,
                                ins=[a2a_in[bi].opt()],
                                outs=[a2a_out[bi].opt()],
                                replica_groups=[list(range(NC))],
                            )

                def qt_chunks(qt):
                    return [
                        (qt, kcb, min(kcb + 8, 4 * qt + 4))
                        for kcb in range(0, 4 * qt + 4, 8)
                    ]

                # software pipeline: emit scores(i+1) before ctx(i) so the PE
                # queue never blocks the ACT exp stream behind ctx matmuls;
                # norms run one more push behind so reciprocals are long done;
                # pair-0 attention interleaves with the QKV q-block loop
                pend = []
                norm_pend = []

                def push(pair, ch):
                    tiles = emit_scores(pair, ch)
                    # only norms queued BEFORE this push (one-unit deferral)
                    ready_norms = norm_pend[:]
                    del norm_pend[:]
                    if pend:
                        emit_ctx(*pend.pop())
                    for n in ready_norms:
                        emit_norm(*n)
                    pend.append((pair, ch, tiles))

                # pair-0 (heads 0,1 together) rides the QKV block loop; its
                # qt3 -- and A2A0/A2A1 -- completes right after QKV(3), so
                # those collectives hide under heads 2-3's attention. Heads
                # 2 and 3 then run as single-head passes: A2A2 fires a full
                # head-pass (~25us) before the end, leaving only A2A3
                # (0.5MB) exposed at the tail.
                for g in range(NQT):
                    emit_qkv(g)
                    for ch in qt_chunks(g):
                        push((0, 1), ch)
                emit_ctx(*pend.pop())
                for qt in range(NQT):
                    for ch in qt_chunks(qt):
                        push((2, 3), ch)
                _fp, _fc, _ft = pend.pop()
                emit_ctx(_fp, _fc, _ft, tail=True)
                for n in norm_pend:
                    emit_norm(*n)
                del norm_pend[:]

                smallp.release()
                pTp.release()
                work.release()

            # ---------- receive + output projection ----------
            # per-head receive for heads 0/1: a2a_out[h] rows = 8 senders x
            # 64 dims; jj blocks of 128 rows pair senders (2jj, 2jj+1), jj
            # 0-1 batch 0, jj 2-3 batch 1
            outp = tc.alloc_tile_pool(name="outp", bufs=1)
            gsrc = a2a_out if coll else a2a_in
            cx = outp.tile([P, 2, 4, TB], BF16, tag="cx")
            tmp = outp.tile([P, 2, TB], BF16, tag="seltmp")
            # ctxf[p, h, jj, t]: wo_sb chunk h*2+jj (Wo rows host-permuted)
            ctxf = outp.tile([P, 2, 2, TB], BF16, tag="ctxf")

            def recv_head(h):
                for hh in range(2):
                    hc = slice(hh * 256, (hh + 1) * 256)
                    nc.sync.dma_start(
                        cx[:, h, :, hc],
                        gsrc[h][:, hc].rearrange("(j q) t -> q j t", q=P),
                    )
                    nc.vector.tensor_scalar(
                        tmp[:, :, hc], cx[:, h, 0:2, hc], bsel_sb[:, 0:1],
                        None, MULT,
                    )
                    nc.vector.scalar_tensor_tensor(
                        ctxf[:, h, :, hc], cx[:, h, 2:4, hc],
                        bsel_sb[:, 1:2], tmp[:, :, hc], MULT, ADD,
                    )

            with tc.tile_pool(name="out_pool", bufs=3) as out_pool:
                o_parts = [
                    outp.tile([P, 512], F32, tag=f"opart{u}", name=f"opart{u}")
                    for u in range(8)
                ]
                # heads 0-1 partials: overlap with the pair-1 A2A
                recv_head(0)
                recv_head(1)
                for u in range(8):
                    tc_i, nt = u // 2, u % 2
                    po = ps_mm.tile([P, 2, 512], F32, tag="mm")
                    for i, (h, jj) in enumerate(
                        [(0, 0), (0, 1), (1, 0), (1, 1)]
                    ):
                        nc.tensor.matmul(
                            po[:, 0, :],
                            ctxf[:, h, jj, tc_i * P : (tc_i + 1) * P],
                            wo_sb[:, h * 2 + jj, nt * 512 : (nt + 1) * 512],
                            start=(i == 0),
                            stop=(i == 3),
                        )
                    nc.vector.tensor_tensor(
                        o_parts[u][:],
                        po[:, 0, :],
                        bo_bc[:, nt * 512 : (nt + 1) * 512],
                        ADD,
                    )
                # keep-warm: dummy matmuls bridging to the pair-1 A2A so the
                # PE doesn't re-throttle (>3us idle drops HAM to half clock)
                # before the pair-1 output projection
                warm = ps_mm.tile([P, 2, 512], F32, tag="mm", name="warm")
                for wj in range(32):
                    nc.tensor.matmul(
                        warm[:, 0, :],
                        ctxf[:, 0, wj % 2, 0:P],
                        wo_sb[:, 0, 0:512],
                        start=True,
                        stop=True,
                    )
                # pair-1 (heads 2+3) merged receive: rows = 8 senders x 128
                # dims; sender blocks j 0-3 batch 0, 4-7 batch 1. Receive,
                # select and project per 128-col token quarter so the first
                # po chains start ~2.5us after the collective lands instead
                # of waiting for the full 1MB readback
                cx1 = outp.tile([P, NC, TB], BF16, tag="cx1")
                tmp1 = outp.tile([P, NQT, TB], BF16, tag="seltmp1")
                ctxf1 = outp.tile([P, NQT, TB], BF16, tag="ctxf1")
                g1 = gsrc[2]
                for tc_i in range(4):
                    hc = slice(tc_i * P, (tc_i + 1) * P)
                    nc.sync.dma_start(
                        cx1[:, :, hc],
                        g1[:, hc].rearrange("(j q) t -> q j t", q=P),
                    )
                    nc.vector.tensor_scalar(
                        tmp1[:, :, hc], cx1[:, 0:4, hc], bsel_sb[:, 0:1],
                        None, MULT,
                    )
                    nc.vector.scalar_tensor_tensor(
                        ctxf1[:, :, hc], cx1[:, 4:8, hc],
                        bsel_sb[:, 1:2], tmp1[:, :, hc], MULT, ADD,
                    )
                    for nt in range(2):
                        u = tc_i * 2 + nt
                        po = ps_mm.tile([P, 2, 512], F32, tag="mm")
                        for g in range(NQT):
                            nc.tensor.matmul(
                                po[:, 0, :],
                                ctxf1[:, g, hc],
                                wo_sb[:, 4 + g, nt * 512 : (nt + 1) * 512],
                                start=(g == 0),
                                stop=(g == NQT - 1),
                            )
                        o_sb = out_pool.tile([P, 512], F32, tag="osb")
                        nc.vector.tensor_tensor(
                            o_sb[:], po[:, 0, :], o_parts[u][:], ADD
                        )
                        nc.sync.dma_start(
                            y_d[
                                tc_i * P : (tc_i + 1) * P,
                                nt * 512 : (nt + 1) * 512,
                            ],
                            o_sb[:],
                        )

            outp.release()

    nc.compile()
    return nc


_NC_CACHE = {}


def _get_nc():
    if "nc" not in _NC_CACHE:
        _NC_CACHE["nc"] = build()
    return _NC_CACHE["nc"]


def _make_in_maps(x, Wq, bq, Wk, bk, Wv, bv, Wo, bo):
    x = np.asarray(x, np.float32)
    Wq, Wk, Wv, Wo = (np.asarray(a, np.float32) for a in (Wq, Wk, Wv, Wo))
    bq, bk, bv, bo = (np.asarray(a, np.float32) for a in (bq, bk, bv, bo))
    np_dx = NP_F8 if FP8_QKV else NP_BF16
    # permute Wo rows to the receive order: chunks 0-3 = per-head (h, jj)
    # layout for heads 0/1; chunks 4-7 = merged pair-1 (sender g) layout
    perm = np.empty(D, np.int64)
    for ci in range(4):
        h, jj = ci // 2, ci % 2
        for par in range(2):
            H = (2 * jj + par) * 4 + h
            base = ci * P + par * HD
            perm[base : base + HD] = np.arange(H * HD, (H + 1) * HD)
    for g in range(4):
        for par in range(2):
            H = 4 * g + 2 + par
            base = (4 + g) * P + par * HD
            perm[base : base + HD] = np.arange(H * HD, (H + 1) * HD)
    wo_b = np.ascontiguousarray(Wo[perm]).astype(NP_BF16)
    in_maps = []
    for c in range(NC):
        b, g = c // 4, c % 4
        sl = slice(g * G, (g + 1) * G)
        bsel = np.tile(
            np.array([1.0 - b, float(b)], np.float32).reshape(1, 2), (P, 1)
        )
        in_maps.append(
            {
                "xt": np.ascontiguousarray(x[b].T).astype(np_dx),
                "wq": np.ascontiguousarray(Wq[:, sl] * WSC).astype(np_dx),
                "wk": np.ascontiguousarray(Wk[:, sl] * WSC).astype(np_dx),
                "wv": np.ascontiguousarray(Wv[:, sl] * WSC).astype(np_dx),
                "xt0": np.ascontiguousarray(x[b].T[:, :TB]).astype(NP_BF16),
                "wq0": np.ascontiguousarray(Wq[:, sl]).astype(NP_BF16),
                "wk0": np.ascontiguousarray(Wk[:, sl]).astype(NP_BF16),
                "wv0": np.ascontiguousarray(Wv[:, sl]).astype(NP_BF16),
                "bq": np.ascontiguousarray(bq[sl].reshape(MC, P).T),
                "bk": np.ascontiguousarray(bk[sl].reshape(MC, P).T),
                "bv": np.ascontiguousarray(bv[sl].reshape(1, G)),
                "wo": wo_b,
                "bo": np.ascontiguousarray(bo.reshape(1, D)),
                "bsel": np.ascontiguousarray(bsel),
            }
        )
    return in_maps


def run(inputs, trace=False, tmpdir=None):
    """Run on 8 cores; returns (output [2,2048,1024], BassKernelResults)."""
    if trace:
        _install_ntff_hook()
    nc = _get_nc()
    in_maps = _make_in_maps(**inputs)
    res = bass_utils.run_bass_kernel_spmd(
        nc, in_maps, core_ids=list(range(NC)), trace=trace, tmpdir=tmpdir
    )
    out = np.empty((B, S, D), np.float32)
    for c in range(NC):
        b, g = c // 4, c % 4
        out[b, g * TB : (g + 1) * TB, :] = res.results[c]["y"]
    return out, res


def kernel(**inputs) -> np.ndarray:
    out, _ = run(inputs, trace=False)
    return out

